# revision 53
# baseline (speedup 1.0000x reference)
# Multi-headed self-attention (B=4, S=2048, D=1024, H=16) on 8 TRN2 NeuronCores.
#
# Sharding: tensor-parallel over heads. Core c computes heads 2c, 2c+1 (=128
# output columns) for all batches. Host pre-transposes x -> xT [D, B*S] and the
# per-core weight slices -> [D, 128] so every matmul contracts over the
# partition dimension. Host gathers the 8 [B*S, 128] outputs into (B,S,D).
#
# Per-core dataflow (bf16 PE operands, fp32 PSUM accumulation):
#   1. Projections (bf16 x, bf16 W): QT/KT [128(2 heads x 64), 8192] bf16 and
#      VT f32, accumulated over 8 d-chunks in PSUM; bias added during the
#      PSUM->SBUF move (DVE per-partition scalar add).
#   2. V2 prep: one packed PE transpose per 128-t chunk turns VT[128(2 heads),
#      128t] into [128t, 128w]; DVE applies the key mask and appends a mask
#      column per head -> v2 chunk layout [V_h0(64)|m|V_h1(64)|m] (130 cols,
#      bf16).
#   3. Attention per (batch, q-block): per k-chunk ONE [128, 1024] PSUM tile
#      holds both heads' scoresT [128 k, 512 q]; the two K=64 bf16 matmuls
#      sit on disjoint PE row halves (partitions 0:64 / 64:128) and their
#      moving streams overlap (~1.4 cols/cycle aggregate, beating the 1
#      col/cycle single-matmul floor). exp alternates engines per k-chunk:
#      ScalarE exact Exp (fused 1/8 scale) for 9/16, VectorE Schraudolph
#      bit-trick exp (one tensor_scalar: bf16 bits = int16(x*A+B), ~1.8% rel
#      err, softmax-safe since the denominator uses the same approximation)
#      for 7/16 — exactly one VectorE chunk per scores-PSUM pair so the two
#      engines always run concurrently. No row-max subtraction (scores std
#      ~0.4, exp is safe, softmax is shift-invariant). Scores/exp run one
#      k-chunk ahead of the PV matmuls (software pipeline). PV matmuls use
#      the 65-col bf16 stationary [V|mask] so the accumulation yields
#      unnormalized h^T plus the softmax denominator. PE-transpose h''^T back
#      to [q, 65], DVE reciprocal of column 64, per-partition scalar
#      multiply, DMA out. hts PSUM->SBUF copies split ScalarE/VectorE.
#   The 0/1 mask is exact this way: reference's exp(-10000) == 0.0 in fp32.
#   Emission interleaves proj/v2-prep of batch b+1 with attention of batch b
#   to keep the PE dense. bf16 quantization of x/W/Q/K/V/probs plus the
#   Schraudolph share puts the end-to-end rel error at ~8e-3 (gate 2e-2).

import sys

import numpy as np

B, S, D, H = 4, 2048, 1024, 16
NC = 8
HPC = H // NC  # heads per core = 2
WH = D // H  # head width = 64
CW = HPC * WH  # per-core output width = 128
BS = B * S  # 8192
DCH = D // 128  # d chunks = 8
QB = S // 512  # q blocks per batch = 4
KCH = S // 128  # k chunks per batch = 16
VCOLS = 2 * (WH + 1)  # v2 chunk cols = 130

# Schraudolph exp in bf16: exp(x*0.125) ~= bitcast_bf16(int16(x*A + Bc))
# (bf16 = 8-bit exp, 7-bit mantissa -> the int domain is 2^7 per octave)
_LN2 = float(np.log(2.0))
SCH_A = 0.125 * (2**7) / _LN2
SCH_B = 127.0 * (2**7) - 5.5
# k-chunks whose exp runs on VectorE (Schraudolph); rest on ScalarE (exact).
DVE_KCS = frozenset({1, 3, 5, 8, 10, 13, 15})

_CACHE = {}


def _ensure_import():
    try:
        import concourse.bass  # noqa: F401
    except ImportError:
        sys.path.insert(0, "/opt/trn_rl_repo")
        import concourse.bass  # noqa: F401


def build_bass():
    if "nc" in _CACHE:
        return _CACHE["nc"]
    _ensure_import()
    import concourse.mybir as mybir
    import concourse.tile as tile
    from concourse import bacc
    from concourse.masks import make_identity

    f32 = mybir.dt.float32
    f32r = mybir.dt.float32r
    bf16 = mybir.dt.bfloat16
    i16 = mybir.dt.int16
    AF = mybir.ActivationFunctionType
    ALU = mybir.AluOpType

    nc = bacc.Bacc(
        "TRN2",
        target_bir_lowering=False,
        debug=False,
        enable_asserts=False,
        num_devices=NC,
    )
    xT_d = nc.dram_tensor("xT", (D, BS), bf16, kind="ExternalInput").ap()
    wq_d = nc.dram_tensor("wqT", (D, CW), bf16, kind="ExternalInput").ap()
    wk_d = nc.dram_tensor("wkT", (D, CW), bf16, kind="ExternalInput").ap()
    wv_d = nc.dram_tensor("wvT", (D, CW), bf16, kind="ExternalInput").ap()
    bq_d = nc.dram_tensor("bq", (CW, 1), f32, kind="ExternalInput").ap()
    bk_d = nc.dram_tensor("bk", (CW, 1), f32, kind="ExternalInput").ap()
    bv_d = nc.dram_tensor("bv", (CW, 1), f32, kind="ExternalInput").ap()
    mask_d = nc.dram_tensor("maskT", (128, B * KCH), f32, kind="ExternalInput").ap()
    out_d = nc.dram_tensor("h_out", (BS, CW), f32, kind="ExternalOutput").ap()

    with tile.TileContext(nc) as tc:
        with (
            tc.tile_pool(name="qkv", bufs=1) as qkv_pool,
            tc.tile_pool(name="xt", bufs=20) as xt_pool,
            tc.tile_pool(name="wsb", bufs=1) as w_pool,
            tc.tile_pool(name="probs", bufs=6) as probs_pool,
            tc.tile_pool(name="v2", bufs=2) as v2_pool,
            tc.tile_pool(name="hts", bufs=4) as hts_pool,
            tc.tile_pool(name="ho", bufs=3) as ho_pool,
            tc.tile_pool(name="rc", bufs=8) as rc_pool,
            tc.tile_pool(name="cst", bufs=1) as cst_pool,
            tc.tile_pool(name="ps_sc", bufs=2, space="PSUM") as ps_sc,
            tc.tile_pool(name="ps_ht", bufs=2, space="PSUM") as ps_ht,
            tc.tile_pool(name="ps_acc", bufs=1, space="PSUM") as ps_acc,
            tc.tile_pool(name="ps_tr", bufs=1, space="PSUM") as ps_tr,
        ):
            ident = cst_pool.tile([128, 128], f32, tag="ident")
            make_identity(nc, ident)



            wsbs = []
            for name, dram in (("wq", wq_d), ("wk", wk_d), ("wv", wv_d)):
                w_sb = w_pool.tile([128, DCH * CW], bf16, tag=name)
                nc.sync.dma_start(
                    out=w_sb.rearrange("p (c w) -> p c w", c=DCH),
                    in_=dram.rearrange("(c p) w -> p c w", p=128),
                )
                wsbs.append(w_sb)
            bsbs = []
            for name, dram in (("bq", bq_d), ("bk", bk_d), ("bv", bv_d)):
                b_sb = cst_pool.tile([128, 1], f32, tag=name)
                nc.sync.dma_start(out=b_sb, in_=dram)
                bsbs.append(b_sb)
            mask_sb = cst_pool.tile([128, B * KCH], f32, tag="mask")
            nc.sync.dma_start(out=mask_sb, in_=mask_d)

            qt = qkv_pool.tile([128, BS], bf16, tag="qt")
            kt = qkv_pool.tile([128, BS], bf16, tag="kt")
            vt = qkv_pool.tile([128, BS], f32, tag="vt")
            qkv_sb = [qt, kt, vt]

            v2_tiles = {}

            def emit_proj_xts(s_):
                xts = []
                for d in range(DCH):
                    xt_t = xt_pool.tile([128, 512], bf16, tag="xt", name=f"xt{s_}_{d}")
                    nc.sync.dma_start(
                        out=xt_t,
                        in_=xT_d[d * 128 : (d + 1) * 128, s_ * 512 : (s_ + 1) * 512],
                    )
                    xts.append(xt_t)
                return xts

            def emit_proj_piece(s_, pi, xts):
                acc = ps_acc.tile([128, 512], f32, tag="acc", name=f"pj{s_}_{pi}")
                w_sb = wsbs[pi]
                for d in range(DCH):
                    nc.tensor.matmul(
                        acc,
                        w_sb[:, d * CW : (d + 1) * CW],
                        xts[d],
                        start=(d == 0),
                        stop=(d == DCH - 1),
                    )
                dst = qkv_sb[pi][:, s_ * 512 : (s_ + 1) * 512]
                nc.vector.tensor_scalar_add(dst, acc, bsbs[pi])

            def emit_proj_sblock(s_):
                xts = emit_proj_xts(s_)
                for pi in range(3):
                    emit_proj_piece(s_, pi, xts)

            def emit_v2_chunk(b, i):
                # One packed transpose: VT[128(2 heads x 64w), 128t] -> [128t, 128w].
                if (b, 0) not in v2_tiles:
                    v2 = v2_pool.tile([128, KCH * VCOLS], bf16, tag="v2", name=f"v2_{b}")
                    v2_tiles[(b, 0)] = v2
                v2 = v2_tiles[(b, 0)]
                vtr_full = ps_acc.tile([128, 512], f32, tag="acc", name=f"vtr{b}_{i}")
                vtr = vtr_full[:, 0:128]
                nc.tensor.transpose(
                    vtr, vt[:, b * S + i * 128 : b * S + (i + 1) * 128], ident
                )
                mcol = mask_sb[:, b * KCH + i : b * KCH + i + 1]
                ch = v2[:, i * VCOLS : (i + 1) * VCOLS]
                ch2 = ch.rearrange("p (g w) -> p g w", g=2)
                vtr2 = vtr.rearrange("p (g w) -> p g w", g=2)
                nc.vector.tensor_scalar_mul(ch2[:, :, 0:WH], vtr2, mcol)
                nc.vector.tensor_copy(ch[:, WH : WH + 1], mcol)
                nc.vector.tensor_copy(ch[:, VCOLS - 1 : VCOLS], mcol)

            def emit_attention_qb(b, qb, extra=()):
                # `extra`: list of (kg_slot, fn) emitted at the top of that kg
                # iteration — used to interleave next-batch proj/v2 PE work so
                # PSUM-drain latencies hide under attention matmuls.
                extra_by_kg = {}
                for slot, fn in extra:
                    extra_by_kg.setdefault(slot, []).append(fn)
                v2 = v2_tiles[(b, 0)]
                base = b * S
                qs = base + qb * 512
                ht0 = ps_ht.tile([65, 512], f32, tag="ht", name=f"ht0_{b}_{qb}")
                ht1 = ps_ht.tile([65, 512], f32, tag="ht", name=f"ht1_{b}_{qb}")
                def emit_scores_exp(kc):
                    sc = ps_sc.tile(
                        [128, 1024], f32, tag="sc", name=f"sc{b}_{qb}_{kc}"
                    )
                    ks = base + kc * 128
                    nc.tensor.matmul(
                        sc[:, 0:512],
                        kt[0:64, ks : ks + 128],
                        qt[0:64, qs : qs + 512],
                        start=True,
                        stop=True,
                    )
                    nc.tensor.matmul(
                        sc[:, 512:1024],
                        kt[64:128, ks : ks + 128],
                        qt[64:128, qs : qs + 512],
                        start=True,
                        stop=True,
                    )
                    pb = probs_pool.tile(
                        [128, 1024], bf16, tag="pb", name=f"pb{b}_{qb}_{kc}"
                    )
                    if kc in DVE_KCS:
                        nc.vector.tensor_scalar(
                            pb.bitcast(i16), sc, SCH_A, SCH_B, ALU.mult, ALU.add
                        )
                    else:
                        nc.scalar.activation(pb, sc, AF.Exp, scale=0.125)
                    return pb

                def emit_pv(kc, pb):
                    c0 = kc * VCOLS
                    nc.tensor.matmul(
                        ht0,
                        v2[:, c0 : c0 + WH + 1],
                        pb[:, 0:512],
                        start=(kc == 0),
                        stop=(kc == KCH - 1),
                        skip_group_check=True,
                    )
                    nc.tensor.matmul(
                        ht1,
                        v2[:, c0 + WH + 1 : c0 + VCOLS],
                        pb[:, 512:1024],
                        start=(kc == 0),
                        stop=(kc == KCH - 1),
                        skip_group_check=True,
                    )

                # Software pipeline: scores/exp run one k-chunk ahead of pv so
                # the PE always has an independent matmul pair while exp runs.
                prev_pb = None
                for kc in range(KCH):
                    if kc % 2 == 0:
                        for fn in extra_by_kg.get(kc // 2, ()):
                            fn()
                    pb = emit_scores_exp(kc)
                    if prev_pb is not None:
                        emit_pv(kc - 1, prev_pb)
                    prev_pb = pb
                emit_pv(KCH - 1, prev_pb)
                hts_t = []
                for hh, ht in ((0, ht0), (1, ht1)):
                    hts = hts_pool.tile(
                        [65, 512], f32, tag="hts", name=f"hts{b}_{qb}_{hh}"
                    )
                    if hh == 0:
                        nc.scalar.copy(hts, ht)
                    else:
                        nc.vector.tensor_copy(hts, ht)
                    hts_t.append(hts)

                # Output epilogue, split into pieces so the two heads can be
                # interleaved (one head's DVE work hides under the other's PE
                # transpose). One [128,288] tr tile per qb = ONE PSUM bank
                # holding four independent 72-col slices (2 per head).
                state = {}

                def epi_piece(hh, t):
                    if "tr" not in state:
                        state["tr"] = ps_tr.tile(
                            [128, 288], f32, tag="tr", name=f"tr{b}_{qb}"
                        )
                        state["ho"] = [
                            ho_pool.tile(
                                [128, 256], f32, tag="ho", name=f"ho{b}_{qb}_{h}"
                            )
                            for h in range(2)
                        ]
                    trt, ho = state["tr"], state["ho"][hh]
                    hts = hts_t[hh]
                    off = hh * 144 + (t % 2) * 72
                    tr2 = trt[:, off : off + 72]
                    nc.tensor.transpose(
                        tr2[:, 0:65],
                        hts[:, t * 128 : (t + 1) * 128],
                        ident[0:65, 0:65],
                    )
                    rc = rc_pool.tile(
                        [128, 1], f32, tag="rc", name=f"rc{b}_{qb}_{hh}_{t}"
                    )
                    nc.vector.reciprocal(rc, tr2[:, 64:65])
                    nc.vector.tensor_scalar_mul(
                        ho[:, t * 64 : (t + 1) * 64], tr2[:, 0:64], rc
                    )
                    if t == 3:
                        hp = hh * WH
                        dst = out_d[qs : qs + 512, hp : hp + 64].rearrange(
                            "(t p) w -> p t w", p=128
                        )
                        nc.gpsimd.dma_start(
                            out=dst, in_=ho.rearrange("p (t w) -> p t w", t=4)
                        )

                return [
                    (lambda hh=hh, t=t: epi_piece(hh, t))
                    for t in range(4)
                    for hh in range(2)
                ]

            # ---- emission: proj/v2 of batch b+1 interleaved with attention(b) ----
            for s in range(4):
                emit_proj_sblock(s)
                for c in range(4 * s, 4 * s + 4):
                    emit_v2_chunk(0, c)
            pending_epis = []
            for b in range(B):
                for qb in range(QB):
                    if b + 1 < B:
                        s_ = 4 * (b + 1) + qb
                        emit_proj_sblock(s_)
                        for c in range(4 * qb, 4 * qb + 4):
                            emit_v2_chunk(b + 1, c)
                    if b + 1 < B:
                        # epilogues run immediately; plenty of proj filler
                        for e in emit_attention_qb(b, qb):
                            e()
                    else:
                        # tail batch: defer the previous qb's output epilogues
                        # into this qb's kc loop as PE filler
                        extra = [(i, e) for i, e in enumerate(pending_epis)]
                        pending_epis = emit_attention_qb(b, qb, extra)
            for e in pending_epis:
                e()

    nc.compile()
    _CACHE["nc"] = nc
    return nc


def make_in_maps(x, mask, Wq, bq, Wk, bk, Wv, bv):
    import ml_dtypes

    bf = ml_dtypes.bfloat16
    x = np.asarray(x, dtype=np.float32)
    xT = np.ascontiguousarray(x.reshape(BS, D).T.astype(bf))
    maskT = np.ascontiguousarray(
        np.asarray(mask, dtype=np.float32)
        .reshape(B, KCH, 128)
        .transpose(2, 0, 1)
        .reshape(128, B * KCH)
    )
    in_maps = []
    for c in range(NC):
        cols = slice(c * CW, (c + 1) * CW)
        in_maps.append(
            {
                "xT": xT,
                "wqT": np.ascontiguousarray(np.asarray(Wq, np.float32)[cols, :].T.astype(bf)),
                "wkT": np.ascontiguousarray(np.asarray(Wk, np.float32)[cols, :].T.astype(bf)),
                "wvT": np.ascontiguousarray(np.asarray(Wv, np.float32)[cols, :].T.astype(bf)),
                "bq": np.ascontiguousarray(np.asarray(bq, np.float32)[cols, None]),
                "bk": np.ascontiguousarray(np.asarray(bk, np.float32)[cols, None]),
                "bv": np.ascontiguousarray(np.asarray(bv, np.float32)[cols, None]),
                "maskT": maskT,
            }
        )
    return in_maps


def assemble(results):
    out = np.empty((BS, D), dtype=np.float32)
    for c in range(NC):
        out[:, c * CW : (c + 1) * CW] = results[c]["h_out"]
    return out.reshape(B, S, D)


def kernel(x, mask, Wq, bq, Wk, bk, Wv, bv, **run_kwargs):
    _ensure_import()
    from concourse.bass_utils import run_bass_kernel_spmd

    nc = build_bass()
    in_maps = make_in_maps(x, mask, Wq, bq, Wk, bk, Wv, bv)
    res = run_bass_kernel_spmd(nc, in_maps, core_ids=list(range(NC)), **run_kwargs)
    _CACHE["last_results"] = res
    return assemble(res.results)


# revision 55
# speedup vs baseline: 1.0065x; 1.0065x over previous
# Multi-headed self-attention (B=4, S=2048, D=1024, H=16) on 8 TRN2 NeuronCores.
#
# Sharding: tensor-parallel over heads. Core c computes heads 2c, 2c+1 (=128
# output columns) for all batches. Host pre-transposes x -> xT [D, B*S] and the
# per-core weight slices -> [D, 128] so every matmul contracts over the
# partition dimension. Host gathers the 8 [B*S, 128] outputs into (B,S,D).
#
# Per-core dataflow (bf16 PE operands, fp32 PSUM accumulation):
#   1. Projections (bf16 x, bf16 W): QT/KT [128(2 heads x 64), 8192] bf16 and
#      VT f32, accumulated over 8 d-chunks in PSUM; bias added during the
#      PSUM->SBUF move (DVE per-partition scalar add).
#   2. V2 prep: one packed PE transpose per 128-t chunk turns VT[128(2 heads),
#      128t] into [128t, 128w]; DVE applies the key mask and appends a mask
#      column per head -> v2 chunk layout [V_h0(64)|m|V_h1(64)|m] (130 cols,
#      bf16).
#   3. Attention per (batch, q-block): per k-chunk ONE [128, 1024] PSUM tile
#      holds both heads' scoresT [128 k, 512 q]; the two K=64 bf16 matmuls
#      sit on disjoint PE row halves (partitions 0:64 / 64:128) and their
#      moving streams overlap (~1.4 cols/cycle aggregate, beating the 1
#      col/cycle single-matmul floor). exp alternates engines per k-chunk:
#      ScalarE exact Exp (fused 1/8 scale) for 9/16, VectorE Schraudolph
#      bit-trick exp (one tensor_scalar: bf16 bits = int16(x*A+B), ~1.8% rel
#      err, softmax-safe since the denominator uses the same approximation)
#      for 7/16 — exactly one VectorE chunk per scores-PSUM pair so the two
#      engines always run concurrently. No row-max subtraction (scores std
#      ~0.4, exp is safe, softmax is shift-invariant). Scores/exp run one
#      k-chunk ahead of the PV matmuls (software pipeline). PV matmuls use
#      the 65-col bf16 stationary [V|mask] so the accumulation yields
#      unnormalized h^T plus the softmax denominator. PE-transpose h''^T back
#      to [q, 65], DVE reciprocal of column 64, per-partition scalar
#      multiply, DMA out. hts PSUM->SBUF copies split ScalarE/VectorE.
#   The 0/1 mask is exact this way: reference's exp(-10000) == 0.0 in fp32.
#   Emission interleaves proj/v2-prep of batch b+1 with attention of batch b
#   to keep the PE dense. bf16 quantization of x/W/Q/K/V/probs plus the
#   Schraudolph share puts the end-to-end rel error at ~8e-3 (gate 2e-2).

import sys

import numpy as np

B, S, D, H = 4, 2048, 1024, 16
NC = 8
HPC = H // NC  # heads per core = 2
WH = D // H  # head width = 64
CW = HPC * WH  # per-core output width = 128
BS = B * S  # 8192
DCH = D // 128  # d chunks = 8
QB = S // 512  # q blocks per batch = 4
KCH = S // 128  # k chunks per batch = 16
VCOLS = 2 * (WH + 1)  # v2 chunk cols = 130

# Schraudolph exp in bf16: exp(x*0.125) ~= bitcast_bf16(int16(x*A + Bc))
# (bf16 = 8-bit exp, 7-bit mantissa -> the int domain is 2^7 per octave)
_LN2 = float(np.log(2.0))
SCH_A = 0.125 * (2**7) / _LN2
SCH_B = 127.0 * (2**7) - 5.5
# k-chunks whose exp runs on VectorE (Schraudolph); rest on ScalarE (exact).
DVE_KCS = frozenset({1, 3, 5, 8, 10, 13, 15})

_CACHE = {}


def _ensure_import():
    try:
        import concourse.bass  # noqa: F401
    except ImportError:
        sys.path.insert(0, "/opt/trn_rl_repo")
        import concourse.bass  # noqa: F401


def build_bass():
    if "nc" in _CACHE:
        return _CACHE["nc"]
    _ensure_import()
    import concourse.mybir as mybir
    import concourse.tile as tile
    from concourse import bacc
    from concourse.masks import make_identity

    f32 = mybir.dt.float32
    f32r = mybir.dt.float32r
    bf16 = mybir.dt.bfloat16
    i16 = mybir.dt.int16
    AF = mybir.ActivationFunctionType
    ALU = mybir.AluOpType

    nc = bacc.Bacc(
        "TRN2",
        target_bir_lowering=False,
        debug=False,
        enable_asserts=False,
        num_devices=NC,
    )
    xT_d = nc.dram_tensor("xT", (D, BS), bf16, kind="ExternalInput").ap()
    wq_d = nc.dram_tensor("wqT", (D, CW), bf16, kind="ExternalInput").ap()
    wk_d = nc.dram_tensor("wkT", (D, CW), bf16, kind="ExternalInput").ap()
    wv_d = nc.dram_tensor("wvT", (D, CW), bf16, kind="ExternalInput").ap()
    bq_d = nc.dram_tensor("bq", (CW, 1), f32, kind="ExternalInput").ap()
    bk_d = nc.dram_tensor("bk", (CW, 1), f32, kind="ExternalInput").ap()
    bv_d = nc.dram_tensor("bv", (CW, 1), f32, kind="ExternalInput").ap()
    mask_d = nc.dram_tensor("maskT", (128, B * KCH), f32, kind="ExternalInput").ap()
    out_d = nc.dram_tensor("h_out", (BS, CW), f32, kind="ExternalOutput").ap()

    with tile.TileContext(nc) as tc:
        with (
            tc.tile_pool(name="qkv", bufs=1) as qkv_pool,
            tc.tile_pool(name="xt", bufs=20) as xt_pool,
            tc.tile_pool(name="wsb", bufs=1) as w_pool,
            tc.tile_pool(name="probs", bufs=6) as probs_pool,
            tc.tile_pool(name="v2", bufs=2) as v2_pool,
            tc.tile_pool(name="hts", bufs=6) as hts_pool,
            tc.tile_pool(name="ho", bufs=4) as ho_pool,
            tc.tile_pool(name="rc", bufs=16) as rc_pool,
            tc.tile_pool(name="cst", bufs=1) as cst_pool,
            tc.tile_pool(name="ps_sc", bufs=2, space="PSUM") as ps_sc,
            tc.tile_pool(name="ps_ht", bufs=2, space="PSUM") as ps_ht,
            tc.tile_pool(name="ps_acc", bufs=1, space="PSUM") as ps_acc,
            tc.tile_pool(name="ps_tr", bufs=1, space="PSUM") as ps_tr,
        ):
            ident = cst_pool.tile([128, 128], f32, tag="ident")
            make_identity(nc, ident)



            wsbs = []
            for name, dram in (("wq", wq_d), ("wk", wk_d), ("wv", wv_d)):
                w_sb = w_pool.tile([128, DCH * CW], bf16, tag=name)
                nc.sync.dma_start(
                    out=w_sb.rearrange("p (c w) -> p c w", c=DCH),
                    in_=dram.rearrange("(c p) w -> p c w", p=128),
                )
                wsbs.append(w_sb)
            bsbs = []
            for name, dram in (("bq", bq_d), ("bk", bk_d), ("bv", bv_d)):
                b_sb = cst_pool.tile([128, 1], f32, tag=name)
                nc.sync.dma_start(out=b_sb, in_=dram)
                bsbs.append(b_sb)
            mask_sb = cst_pool.tile([128, B * KCH], f32, tag="mask")
            nc.sync.dma_start(out=mask_sb, in_=mask_d)

            qt = qkv_pool.tile([128, BS], bf16, tag="qt")
            kt = qkv_pool.tile([128, BS], bf16, tag="kt")
            vt = qkv_pool.tile([128, BS], f32, tag="vt")
            qkv_sb = [qt, kt, vt]

            v2_tiles = {}

            def emit_proj_xts(s_):
                xts = []
                for d in range(DCH):
                    xt_t = xt_pool.tile([128, 512], bf16, tag="xt", name=f"xt{s_}_{d}")
                    nc.sync.dma_start(
                        out=xt_t,
                        in_=xT_d[d * 128 : (d + 1) * 128, s_ * 512 : (s_ + 1) * 512],
                    )
                    xts.append(xt_t)
                return xts

            def emit_proj_piece(s_, pi, xts):
                acc = ps_acc.tile([128, 512], f32, tag="acc", name=f"pj{s_}_{pi}")
                w_sb = wsbs[pi]
                for d in range(DCH):
                    nc.tensor.matmul(
                        acc,
                        w_sb[:, d * CW : (d + 1) * CW],
                        xts[d],
                        start=(d == 0),
                        stop=(d == DCH - 1),
                    )
                dst = qkv_sb[pi][:, s_ * 512 : (s_ + 1) * 512]
                nc.vector.tensor_scalar_add(dst, acc, bsbs[pi])

            def emit_proj_sblock(s_):
                xts = emit_proj_xts(s_)
                for pi in range(3):
                    emit_proj_piece(s_, pi, xts)

            def emit_v2_chunk(b, i):
                # One packed transpose: VT[128(2 heads x 64w), 128t] -> [128t, 128w].
                if (b, 0) not in v2_tiles:
                    v2 = v2_pool.tile([128, KCH * VCOLS], bf16, tag="v2", name=f"v2_{b}")
                    v2_tiles[(b, 0)] = v2
                v2 = v2_tiles[(b, 0)]
                vtr_full = ps_acc.tile([128, 512], f32, tag="acc", name=f"vtr{b}_{i}")
                vtr = vtr_full[:, 0:128]
                nc.tensor.transpose(
                    vtr, vt[:, b * S + i * 128 : b * S + (i + 1) * 128], ident
                )
                mcol = mask_sb[:, b * KCH + i : b * KCH + i + 1]
                ch = v2[:, i * VCOLS : (i + 1) * VCOLS]
                ch2 = ch.rearrange("p (g w) -> p g w", g=2)
                vtr2 = vtr.rearrange("p (g w) -> p g w", g=2)
                nc.vector.tensor_scalar_mul(ch2[:, :, 0:WH], vtr2, mcol)
                nc.vector.tensor_copy(ch[:, WH : WH + 1], mcol)
                nc.vector.tensor_copy(ch[:, VCOLS - 1 : VCOLS], mcol)

            def emit_attention_qb(b, qb, extra=()):
                # `extra`: list of (kg_slot, fn) emitted at the top of that kg
                # iteration — used to interleave next-batch proj/v2 PE work so
                # PSUM-drain latencies hide under attention matmuls.
                extra_by_kg = {}
                for slot, fn in extra:
                    extra_by_kg.setdefault(slot, []).append(fn)
                v2 = v2_tiles[(b, 0)]
                base = b * S
                qs = base + qb * 512
                ht0 = ps_ht.tile([65, 512], f32, tag="ht", name=f"ht0_{b}_{qb}")
                ht1 = ps_ht.tile([65, 512], f32, tag="ht", name=f"ht1_{b}_{qb}")
                def emit_scores_exp(kc):
                    sc = ps_sc.tile(
                        [128, 1024], f32, tag="sc", name=f"sc{b}_{qb}_{kc}"
                    )
                    ks = base + kc * 128
                    nc.tensor.matmul(
                        sc[:, 0:512],
                        kt[0:64, ks : ks + 128],
                        qt[0:64, qs : qs + 512],
                        start=True,
                        stop=True,
                    )
                    nc.tensor.matmul(
                        sc[:, 512:1024],
                        kt[64:128, ks : ks + 128],
                        qt[64:128, qs : qs + 512],
                        start=True,
                        stop=True,
                    )
                    pb = probs_pool.tile(
                        [128, 1024], bf16, tag="pb", name=f"pb{b}_{qb}_{kc}"
                    )
                    if b == B - 1:
                        # tail batch has no proj filler; halve the exp chain
                        # latency by splitting each tile over both engines
                        nc.scalar.activation(
                            pb[:, 0:512], sc[:, 0:512], AF.Exp, scale=0.125
                        )
                        nc.vector.tensor_scalar(
                            pb[:, 512:1024].bitcast(i16),
                            sc[:, 512:1024],
                            SCH_A,
                            SCH_B,
                            ALU.mult,
                            ALU.add,
                        )
                    elif kc in DVE_KCS:
                        nc.vector.tensor_scalar(
                            pb.bitcast(i16), sc, SCH_A, SCH_B, ALU.mult, ALU.add
                        )
                    else:
                        nc.scalar.activation(pb, sc, AF.Exp, scale=0.125)
                    return pb

                def emit_pv(kc, pb):
                    c0 = kc * VCOLS
                    nc.tensor.matmul(
                        ht0,
                        v2[:, c0 : c0 + WH + 1],
                        pb[:, 0:512],
                        start=(kc == 0),
                        stop=(kc == KCH - 1),
                        skip_group_check=True,
                    )
                    nc.tensor.matmul(
                        ht1,
                        v2[:, c0 + WH + 1 : c0 + VCOLS],
                        pb[:, 512:1024],
                        start=(kc == 0),
                        stop=(kc == KCH - 1),
                        skip_group_check=True,
                    )

                # Software pipeline: scores/exp run one k-chunk ahead of pv so
                # the PE always has an independent matmul pair while exp runs.
                prev_pb = None
                for kc in range(KCH):
                    if kc % 2 == 0:
                        for fn in extra_by_kg.get(kc // 2, ()):
                            fn()
                    pb = emit_scores_exp(kc)
                    if prev_pb is not None:
                        emit_pv(kc - 1, prev_pb)
                    prev_pb = pb
                emit_pv(KCH - 1, prev_pb)
                hts_t = []
                for hh, ht in ((0, ht0), (1, ht1)):
                    hts = hts_pool.tile(
                        [65, 512], f32, tag="hts", name=f"hts{b}_{qb}_{hh}"
                    )
                    if hh == 0:
                        nc.scalar.copy(hts, ht)
                    else:
                        nc.vector.tensor_copy(hts, ht)
                    hts_t.append(hts)

                # Output epilogue, split into pieces so the two heads can be
                # interleaved (one head's DVE work hides under the other's PE
                # transpose). One [128,288] tr tile per qb = ONE PSUM bank
                # holding four independent 72-col slices (2 per head).
                state = {}

                def epi_piece(hh, t):
                    if "tr" not in state:
                        state["tr"] = ps_tr.tile(
                            [128, 288], f32, tag="tr", name=f"tr{b}_{qb}"
                        )
                        state["ho"] = [
                            ho_pool.tile(
                                [128, 256], f32, tag="ho", name=f"ho{b}_{qb}_{h}"
                            )
                            for h in range(2)
                        ]
                    trt, ho = state["tr"], state["ho"][hh]
                    hts = hts_t[hh]
                    off = hh * 144 + (t % 2) * 72
                    tr2 = trt[:, off : off + 72]
                    nc.tensor.transpose(
                        tr2[:, 0:65],
                        hts[:, t * 128 : (t + 1) * 128],
                        ident[0:65, 0:65],
                    )
                    rc = rc_pool.tile(
                        [128, 1], f32, tag="rc", name=f"rc{b}_{qb}_{hh}_{t}"
                    )
                    nc.vector.reciprocal(rc, tr2[:, 64:65])
                    nc.vector.tensor_scalar_mul(
                        ho[:, t * 64 : (t + 1) * 64], tr2[:, 0:64], rc
                    )
                    if t == 3:
                        hp = hh * WH
                        dst = out_d[qs : qs + 512, hp : hp + 64].rearrange(
                            "(t p) w -> p t w", p=128
                        )
                        nc.gpsimd.dma_start(
                            out=dst, in_=ho.rearrange("p (t w) -> p t w", t=4)
                        )

                return [
                    (lambda hh=hh, t=t: epi_piece(hh, t))
                    for t in range(4)
                    for hh in range(2)
                ]

            # ---- emission: proj/v2 of batch b+1 interleaved with attention(b) ----
            for s in range(4):
                emit_proj_sblock(s)
                for c in range(4 * s, 4 * s + 4):
                    emit_v2_chunk(0, c)
            pending_epis = []
            for b in range(B):
                for qb in range(QB):
                    if b + 1 < B:
                        s_ = 4 * (b + 1) + qb
                        emit_proj_sblock(s_)
                        for c in range(4 * qb, 4 * qb + 4):
                            emit_v2_chunk(b + 1, c)
                    if b + 1 < B:
                        # epilogues run immediately; plenty of proj filler
                        for e in emit_attention_qb(b, qb):
                            e()
                    else:
                        # tail batch: defer the previous qb's output epilogues
                        # into this qb's kc loop as PE filler
                        extra = [(i, e) for i, e in enumerate(pending_epis)]
                        pending_epis = emit_attention_qb(b, qb, extra)
            for e in pending_epis:
                e()

    nc.compile()
    _CACHE["nc"] = nc
    return nc


def make_in_maps(x, mask, Wq, bq, Wk, bk, Wv, bv):
    import ml_dtypes

    bf = ml_dtypes.bfloat16
    x = np.asarray(x, dtype=np.float32)
    xT = np.ascontiguousarray(x.reshape(BS, D).T.astype(bf))
    maskT = np.ascontiguousarray(
        np.asarray(mask, dtype=np.float32)
        .reshape(B, KCH, 128)
        .transpose(2, 0, 1)
        .reshape(128, B * KCH)
    )
    in_maps = []
    for c in range(NC):
        cols = slice(c * CW, (c + 1) * CW)
        in_maps.append(
            {
                "xT": xT,
                "wqT": np.ascontiguousarray(np.asarray(Wq, np.float32)[cols, :].T.astype(bf)),
                "wkT": np.ascontiguousarray(np.asarray(Wk, np.float32)[cols, :].T.astype(bf)),
                "wvT": np.ascontiguousarray(np.asarray(Wv, np.float32)[cols, :].T.astype(bf)),
                "bq": np.ascontiguousarray(np.asarray(bq, np.float32)[cols, None]),
                "bk": np.ascontiguousarray(np.asarray(bk, np.float32)[cols, None]),
                "bv": np.ascontiguousarray(np.asarray(bv, np.float32)[cols, None]),
                "maskT": maskT,
            }
        )
    return in_maps


def assemble(results):
    out = np.empty((BS, D), dtype=np.float32)
    for c in range(NC):
        out[:, c * CW : (c + 1) * CW] = results[c]["h_out"]
    return out.reshape(B, S, D)


def kernel(x, mask, Wq, bq, Wk, bk, Wv, bv, **run_kwargs):
    _ensure_import()
    from concourse.bass_utils import run_bass_kernel_spmd

    nc = build_bass()
    in_maps = make_in_maps(x, mask, Wq, bq, Wk, bk, Wv, bv)
    res = run_bass_kernel_spmd(nc, in_maps, core_ids=list(range(NC)), **run_kwargs)
    _CACHE["last_results"] = res
    return assemble(res.results)


# revision 56
# speedup vs baseline: 1.0410x; 1.0343x over previous
# Multi-headed self-attention (B=4, S=2048, D=1024, H=16) on 8 TRN2 NeuronCores.
#
# Sharding: tensor-parallel over heads. Core c computes heads 2c, 2c+1 (=128
# output columns) for all batches. Host pre-transposes x -> xT [D, B*S] and the
# per-core weight slices -> [D, 128] so every matmul contracts over the
# partition dimension. Host gathers the 8 [B*S, 128] outputs into (B,S,D).
#
# Per-core dataflow (bf16 PE operands, fp32 PSUM accumulation):
#   1. Projections (bf16 x, bf16 W): QT/KT [128(2 heads x 64), 8192] bf16 and
#      VT f32, accumulated over 8 d-chunks in PSUM; bias added during the
#      PSUM->SBUF move (DVE per-partition scalar add).
#   2. V2 prep: one packed PE transpose per 128-t chunk turns VT[128(2 heads),
#      128t] into [128t, 128w]; DVE applies the key mask and appends a mask
#      column per head -> v2 chunk layout [V_h0(64)|m|V_h1(64)|m] (130 cols,
#      bf16).
#   3. Attention per (batch, q-block): per k-chunk ONE [128, 1024] PSUM tile
#      holds both heads' scoresT [128 k, 512 q]; the two K=64 bf16 matmuls
#      sit on disjoint PE row halves (partitions 0:64 / 64:128) and their
#      moving streams overlap (~1.4 cols/cycle aggregate, beating the 1
#      col/cycle single-matmul floor). exp alternates engines per k-chunk:
#      ScalarE exact Exp (fused 1/8 scale) for 9/16, VectorE Schraudolph
#      bit-trick exp (one tensor_scalar: bf16 bits = int16(x*A+B), ~1.8% rel
#      err, softmax-safe since the denominator uses the same approximation)
#      for 7/16 — exactly one VectorE chunk per scores-PSUM pair so the two
#      engines always run concurrently. No row-max subtraction (scores std
#      ~0.4, exp is safe, softmax is shift-invariant). Scores/exp run one
#      k-chunk ahead of the PV matmuls (software pipeline). PV matmuls use
#      the 65-col bf16 stationary [V|mask] so the accumulation yields
#      unnormalized h^T plus the softmax denominator. PE-transpose h''^T back
#      to [q, 65], DVE reciprocal of column 64, per-partition scalar
#      multiply, DMA out. hts PSUM->SBUF copies split ScalarE/VectorE.
#   The 0/1 mask is exact this way: reference's exp(-10000) == 0.0 in fp32.
#   Emission interleaves proj/v2-prep of batch b+1 with attention of batch b
#   to keep the PE dense. bf16 quantization of x/W/Q/K/V/probs plus the
#   Schraudolph share puts the end-to-end rel error at ~8e-3 (gate 2e-2).

import sys

import numpy as np

B, S, D, H = 4, 2048, 1024, 16
NC = 8
HPC = H // NC  # heads per core = 2
WH = D // H  # head width = 64
CW = HPC * WH  # per-core output width = 128
BS = B * S  # 8192
DCH = D // 128  # d chunks = 8
QB = S // 512  # q blocks per batch = 4
KCH = S // 128  # k chunks per batch = 16
VCOLS = 2 * (WH + 1)  # v2 chunk cols = 130

# Schraudolph exp in bf16: exp(x*0.125) ~= bitcast_bf16(int16(x*A + Bc))
# (bf16 = 8-bit exp, 7-bit mantissa -> the int domain is 2^7 per octave)
_LN2 = float(np.log(2.0))
SCH_A = 0.125 * (2**7) / _LN2
SCH_B = 127.0 * (2**7) - 5.5
# k-chunks whose exp runs on VectorE (Schraudolph); rest on ScalarE (exact).
DVE_KCS = frozenset({1, 3, 5, 8, 10, 13, 15})

_CACHE = {}


def _ensure_import():
    try:
        import concourse.bass  # noqa: F401
    except ImportError:
        sys.path.insert(0, "/opt/trn_rl_repo")
        import concourse.bass  # noqa: F401


def build_bass():
    if "nc" in _CACHE:
        return _CACHE["nc"]
    _ensure_import()
    import concourse.mybir as mybir
    import concourse.tile as tile
    from concourse import bacc
    from concourse.masks import make_identity

    f32 = mybir.dt.float32
    f32r = mybir.dt.float32r
    bf16 = mybir.dt.bfloat16
    i16 = mybir.dt.int16
    AF = mybir.ActivationFunctionType
    ALU = mybir.AluOpType

    nc = bacc.Bacc(
        "TRN2",
        target_bir_lowering=False,
        debug=False,
        enable_asserts=False,
        num_devices=NC,
    )
    xT_d = nc.dram_tensor("xT", (D, BS), bf16, kind="ExternalInput").ap()
    wq_d = nc.dram_tensor("wqT", (D, CW), bf16, kind="ExternalInput").ap()
    wk_d = nc.dram_tensor("wkT", (D, CW), bf16, kind="ExternalInput").ap()
    wv_d = nc.dram_tensor("wvT", (D, CW), bf16, kind="ExternalInput").ap()
    bq_d = nc.dram_tensor("bq", (CW, 1), f32, kind="ExternalInput").ap()
    bk_d = nc.dram_tensor("bk", (CW, 1), f32, kind="ExternalInput").ap()
    bv_d = nc.dram_tensor("bv", (CW, 1), f32, kind="ExternalInput").ap()
    mask_d = nc.dram_tensor("maskT", (128, B * KCH), f32, kind="ExternalInput").ap()
    out_d = nc.dram_tensor("h_out", (BS, CW), f32, kind="ExternalOutput").ap()

    with tile.TileContext(nc) as tc:
        with (
            tc.tile_pool(name="qkv", bufs=1) as qkv_pool,
            tc.tile_pool(name="xt", bufs=20) as xt_pool,
            tc.tile_pool(name="wsb", bufs=1) as w_pool,
            tc.tile_pool(name="probs", bufs=6) as probs_pool,
            tc.tile_pool(name="v2", bufs=2) as v2_pool,
            tc.tile_pool(name="hts", bufs=6) as hts_pool,
            tc.tile_pool(name="ho", bufs=4) as ho_pool,
            tc.tile_pool(name="rc", bufs=16) as rc_pool,
            tc.tile_pool(name="cst", bufs=1) as cst_pool,
            tc.tile_pool(name="ps_sc", bufs=2, space="PSUM") as ps_sc,
            tc.tile_pool(name="ps_ht", bufs=2, space="PSUM") as ps_ht,
            tc.tile_pool(name="ps_acc", bufs=1, space="PSUM") as ps_acc,
            tc.tile_pool(name="ps_tr", bufs=1, space="PSUM") as ps_tr,
        ):
            ident = cst_pool.tile([128, 128], f32, tag="ident")
            make_identity(nc, ident)



            wsbs = []
            for name, dram in (("wq", wq_d), ("wk", wk_d), ("wv", wv_d)):
                w_sb = w_pool.tile([128, DCH * CW], bf16, tag=name)
                nc.sync.dma_start(
                    out=w_sb.rearrange("p (c w) -> p c w", c=DCH),
                    in_=dram.rearrange("(c p) w -> p c w", p=128),
                )
                wsbs.append(w_sb)
            bsbs = []
            for name, dram in (("bq", bq_d), ("bk", bk_d), ("bv", bv_d)):
                b_sb = cst_pool.tile([128, 1], f32, tag=name)
                nc.sync.dma_start(out=b_sb, in_=dram)
                bsbs.append(b_sb)
            mask_sb = cst_pool.tile([128, B * KCH], f32, tag="mask")
            nc.sync.dma_start(out=mask_sb, in_=mask_d)

            qt = qkv_pool.tile([128, BS], bf16, tag="qt")
            kt = qkv_pool.tile([128, BS], bf16, tag="kt")
            vt = qkv_pool.tile([128, BS], f32, tag="vt")
            qkv_sb = [qt, kt, vt]

            v2_tiles = {}

            def emit_proj_xts(s_):
                xts = []
                for d in range(DCH):
                    xt_t = xt_pool.tile([128, 512], bf16, tag="xt", name=f"xt{s_}_{d}")
                    nc.sync.dma_start(
                        out=xt_t,
                        in_=xT_d[d * 128 : (d + 1) * 128, s_ * 512 : (s_ + 1) * 512],
                    )
                    xts.append(xt_t)
                return xts

            def emit_proj_piece(s_, pi, xts):
                acc = ps_acc.tile([128, 512], f32, tag="acc", name=f"pj{s_}_{pi}")
                w_sb = wsbs[pi]
                for d in range(DCH):
                    nc.tensor.matmul(
                        acc,
                        w_sb[:, d * CW : (d + 1) * CW],
                        xts[d],
                        start=(d == 0),
                        stop=(d == DCH - 1),
                    )
                dst = qkv_sb[pi][:, s_ * 512 : (s_ + 1) * 512]
                nc.vector.tensor_scalar_add(dst, acc, bsbs[pi])

            def emit_proj_sblock(s_):
                xts = emit_proj_xts(s_)
                for pi in range(3):
                    emit_proj_piece(s_, pi, xts)

            def emit_v2_chunk(b, i):
                # One packed transpose: VT[128(2 heads x 64w), 128t] -> [128t, 128w].
                if (b, 0) not in v2_tiles:
                    v2 = v2_pool.tile([128, KCH * VCOLS], bf16, tag="v2", name=f"v2_{b}")
                    v2_tiles[(b, 0)] = v2
                v2 = v2_tiles[(b, 0)]
                vtr_full = ps_acc.tile([128, 512], f32, tag="acc", name=f"vtr{b}_{i}")
                vtr = vtr_full[:, 0:128]
                nc.tensor.transpose(
                    vtr, vt[:, b * S + i * 128 : b * S + (i + 1) * 128], ident
                )
                mcol = mask_sb[:, b * KCH + i : b * KCH + i + 1]
                ch = v2[:, i * VCOLS : (i + 1) * VCOLS]
                ch2 = ch.rearrange("p (g w) -> p g w", g=2)
                vtr2 = vtr.rearrange("p (g w) -> p g w", g=2)
                nc.vector.tensor_scalar_mul(ch2[:, :, 0:WH], vtr2, mcol)
                nc.vector.tensor_copy(ch[:, WH : WH + 1], mcol)
                nc.vector.tensor_copy(ch[:, VCOLS - 1 : VCOLS], mcol)

            def emit_attention_qb(b, qb, extra=()):
                # `extra`: list of (kg_slot, fn) emitted at the top of that kg
                # iteration — used to interleave next-batch proj/v2 PE work so
                # PSUM-drain latencies hide under attention matmuls.
                extra_by_kg = {}
                for slot, fn in extra:
                    extra_by_kg.setdefault(slot, []).append(fn)
                v2 = v2_tiles[(b, 0)]
                base = b * S
                qs = base + qb * 512
                ht0 = ps_ht.tile([65, 512], f32, tag="ht", name=f"ht0_{b}_{qb}")
                ht1 = ps_ht.tile([65, 512], f32, tag="ht", name=f"ht1_{b}_{qb}")
                def emit_scores_exp(kc):
                    sc = ps_sc.tile(
                        [128, 1024], f32, tag="sc", name=f"sc{b}_{qb}_{kc}"
                    )
                    ks = base + kc * 128
                    nc.tensor.matmul(
                        sc[:, 0:512],
                        kt[0:64, ks : ks + 128],
                        qt[0:64, qs : qs + 512],
                        start=True,
                        stop=True,
                    )
                    nc.tensor.matmul(
                        sc[:, 512:1024],
                        kt[64:128, ks : ks + 128],
                        qt[64:128, qs : qs + 512],
                        start=True,
                        stop=True,
                    )
                    pb = probs_pool.tile(
                        [128, 1024], bf16, tag="pb", name=f"pb{b}_{qb}_{kc}"
                    )
                    if b == B - 1:
                        # tail batch has no proj filler; halve the exp chain
                        # latency by splitting each tile over both engines
                        nc.scalar.activation(
                            pb[:, 0:512], sc[:, 0:512], AF.Exp, scale=0.125
                        )
                        nc.vector.tensor_scalar(
                            pb[:, 512:1024].bitcast(i16),
                            sc[:, 512:1024],
                            SCH_A,
                            SCH_B,
                            ALU.mult,
                            ALU.add,
                        )
                    elif kc in DVE_KCS:
                        nc.vector.tensor_scalar(
                            pb.bitcast(i16), sc, SCH_A, SCH_B, ALU.mult, ALU.add
                        )
                    else:
                        nc.scalar.activation(pb, sc, AF.Exp, scale=0.125)
                    return pb

                def emit_pv(kc, pb):
                    c0 = kc * VCOLS
                    nc.tensor.matmul(
                        ht0,
                        v2[:, c0 : c0 + WH + 1],
                        pb[:, 0:512],
                        start=(kc == 0),
                        stop=(kc == KCH - 1),
                        skip_group_check=True,
                    )
                    nc.tensor.matmul(
                        ht1,
                        v2[:, c0 + WH + 1 : c0 + VCOLS],
                        pb[:, 512:1024],
                        start=(kc == 0),
                        stop=(kc == KCH - 1),
                        skip_group_check=True,
                    )

                # Software pipeline: scores/exp run one k-chunk ahead of pv so
                # the PE always has an independent matmul pair while exp runs.
                prev_pb = None
                for kc in range(KCH):
                    if kc % 2 == 0:
                        for fn in extra_by_kg.get(kc // 2, ()):
                            fn()
                    pb = emit_scores_exp(kc)
                    if prev_pb is not None:
                        emit_pv(kc - 1, prev_pb)
                    prev_pb = pb
                emit_pv(KCH - 1, prev_pb)
                hts_t = []
                for hh, ht in ((0, ht0), (1, ht1)):
                    hts = hts_pool.tile(
                        [65, 512], f32, tag="hts", name=f"hts{b}_{qb}_{hh}"
                    )
                    if hh == 0:
                        nc.scalar.copy(hts, ht)
                    else:
                        nc.vector.tensor_copy(hts, ht)
                    hts_t.append(hts)

                # Output epilogue, split into pieces so the two heads can be
                # interleaved (one head's DVE work hides under the other's PE
                # transpose). One [128,288] tr tile per qb = ONE PSUM bank
                # holding four independent 72-col slices (2 per head).
                state = {}

                def epi_piece(hh, t):
                    if "tr" not in state:
                        state["tr"] = ps_tr.tile(
                            [128, 288], f32, tag="tr", name=f"tr{b}_{qb}"
                        )
                        state["ho"] = [
                            ho_pool.tile(
                                [128, 256], f32, tag="ho", name=f"ho{b}_{qb}_{h}"
                            )
                            for h in range(2)
                        ]
                    trt, ho = state["tr"], state["ho"][hh]
                    hts = hts_t[hh]
                    off = hh * 144 + (t % 2) * 72
                    tr2 = trt[:, off : off + 72]
                    nc.tensor.transpose(
                        tr2[:, 0:65],
                        hts[:, t * 128 : (t + 1) * 128],
                        ident[0:65, 0:65],
                    )
                    rc = rc_pool.tile(
                        [128, 1], f32, tag="rc", name=f"rc{b}_{qb}_{hh}_{t}"
                    )
                    nc.vector.reciprocal(rc, tr2[:, 64:65])
                    nc.vector.tensor_scalar_mul(
                        ho[:, t * 64 : (t + 1) * 64], tr2[:, 0:64], rc
                    )
                    if t == 3:
                        hp = hh * WH
                        dst = out_d[qs : qs + 512, hp : hp + 64].rearrange(
                            "(t p) w -> p t w", p=128
                        )
                        nc.gpsimd.dma_start(
                            out=dst, in_=ho.rearrange("p (t w) -> p t w", t=4)
                        )

                return [
                    (lambda hh=hh, t=t: epi_piece(hh, t))
                    for t in range(4)
                    for hh in range(2)
                ]

            # ---- emission: proj/v2 of batch b+1 interleaved with attention(b) ----
            for s in range(4):
                emit_proj_sblock(s)
                for c in range(4 * s, 4 * s + 4):
                    emit_v2_chunk(0, c)
            pending_epis = []
            for b in range(B):
                for qb in range(QB):
                    if b + 1 < B:
                        s_ = 4 * (b + 1) + qb
                        emit_proj_sblock(s_)
                        for c in range(4 * qb, 4 * qb + 4):
                            emit_v2_chunk(b + 1, c)
                    # defer the previous qb's output epilogues into this qb's
                    # kc loop: the PE transposes then overlap attention instead
                    # of stalling on the cross-engine hts-copy latency
                    extra = [(i, e) for i, e in enumerate(pending_epis)]
                    pending_epis = emit_attention_qb(b, qb, extra)
            for e in pending_epis:
                e()

    nc.compile()
    _CACHE["nc"] = nc
    return nc


def make_in_maps(x, mask, Wq, bq, Wk, bk, Wv, bv):
    import ml_dtypes

    bf = ml_dtypes.bfloat16
    x = np.asarray(x, dtype=np.float32)
    xT = np.ascontiguousarray(x.reshape(BS, D).T.astype(bf))
    maskT = np.ascontiguousarray(
        np.asarray(mask, dtype=np.float32)
        .reshape(B, KCH, 128)
        .transpose(2, 0, 1)
        .reshape(128, B * KCH)
    )
    in_maps = []
    for c in range(NC):
        cols = slice(c * CW, (c + 1) * CW)
        in_maps.append(
            {
                "xT": xT,
                "wqT": np.ascontiguousarray(np.asarray(Wq, np.float32)[cols, :].T.astype(bf)),
                "wkT": np.ascontiguousarray(np.asarray(Wk, np.float32)[cols, :].T.astype(bf)),
                "wvT": np.ascontiguousarray(np.asarray(Wv, np.float32)[cols, :].T.astype(bf)),
                "bq": np.ascontiguousarray(np.asarray(bq, np.float32)[cols, None]),
                "bk": np.ascontiguousarray(np.asarray(bk, np.float32)[cols, None]),
                "bv": np.ascontiguousarray(np.asarray(bv, np.float32)[cols, None]),
                "maskT": maskT,
            }
        )
    return in_maps


def assemble(results):
    out = np.empty((BS, D), dtype=np.float32)
    for c in range(NC):
        out[:, c * CW : (c + 1) * CW] = results[c]["h_out"]
    return out.reshape(B, S, D)


def kernel(x, mask, Wq, bq, Wk, bk, Wv, bv, **run_kwargs):
    _ensure_import()
    from concourse.bass_utils import run_bass_kernel_spmd

    nc = build_bass()
    in_maps = make_in_maps(x, mask, Wq, bq, Wk, bk, Wv, bv)
    res = run_bass_kernel_spmd(nc, in_maps, core_ids=list(range(NC)), **run_kwargs)
    _CACHE["last_results"] = res
    return assemble(res.results)


# revision 57
# speedup vs baseline: 1.0429x; 1.0019x over previous
# Multi-headed self-attention (B=4, S=2048, D=1024, H=16) on 8 TRN2 NeuronCores.
#
# Sharding: tensor-parallel over heads. Core c computes heads 2c, 2c+1 (=128
# output columns) for all batches. Host pre-transposes x -> xT [D, B*S] and the
# per-core weight slices -> [D, 128] so every matmul contracts over the
# partition dimension. Host gathers the 8 [B*S, 128] outputs into (B,S,D).
#
# Per-core dataflow (bf16 PE operands, fp32 PSUM accumulation):
#   1. Projections (bf16 x, bf16 W): QT/KT [128(2 heads x 64), 8192] bf16 and
#      VT f32, accumulated over 8 d-chunks in PSUM; bias added during the
#      PSUM->SBUF move (DVE per-partition scalar add).
#   2. V2 prep: one packed PE transpose per 128-t chunk turns VT[128(2 heads),
#      128t] into [128t, 128w]; DVE applies the key mask and appends a mask
#      column per head -> v2 chunk layout [V_h0(64)|m|V_h1(64)|m] (130 cols,
#      bf16).
#   3. Attention per (batch, q-block): per k-chunk ONE [128, 1024] PSUM tile
#      holds both heads' scoresT [128 k, 512 q]; the two K=64 bf16 matmuls
#      sit on disjoint PE row halves (partitions 0:64 / 64:128) and their
#      moving streams overlap (~1.4 cols/cycle aggregate, beating the 1
#      col/cycle single-matmul floor). exp alternates engines per k-chunk:
#      ScalarE exact Exp (fused 1/8 scale) for 9/16, VectorE Schraudolph
#      bit-trick exp (one tensor_scalar: bf16 bits = int16(x*A+B), ~1.8% rel
#      err, softmax-safe since the denominator uses the same approximation)
#      for 7/16 — exactly one VectorE chunk per scores-PSUM pair so the two
#      engines always run concurrently. No row-max subtraction (scores std
#      ~0.4, exp is safe, softmax is shift-invariant). Scores/exp run one
#      k-chunk ahead of the PV matmuls (software pipeline). PV matmuls use
#      the 65-col bf16 stationary [V|mask] so the accumulation yields
#      unnormalized h^T plus the softmax denominator. PE-transpose h''^T back
#      to [q, 65], DVE reciprocal of column 64, per-partition scalar
#      multiply, DMA out. hts PSUM->SBUF copies split ScalarE/VectorE.
#   The 0/1 mask is exact this way: reference's exp(-10000) == 0.0 in fp32.
#   Emission interleaves proj/v2-prep of batch b+1 with attention of batch b
#   to keep the PE dense; each q-block's output epilogue (transpose/normalize/
#   DMA) is deferred into the NEXT q-block's kc loop so the PE transposes
#   overlap attention instead of stalling on cross-engine copy latency.
#   bf16 quantization of x/W/Q/K/V/probs plus the Schraudolph share puts the
#   end-to-end rel error at ~8e-3 (gate 2e-2).

import sys

import numpy as np

B, S, D, H = 4, 2048, 1024, 16
NC = 8
HPC = H // NC  # heads per core = 2
WH = D // H  # head width = 64
CW = HPC * WH  # per-core output width = 128
BS = B * S  # 8192
DCH = D // 128  # d chunks = 8
QB = S // 512  # q blocks per batch = 4
KCH = S // 128  # k chunks per batch = 16
VCOLS = 2 * (WH + 1)  # v2 chunk cols = 130

# Schraudolph exp in bf16: exp(x*0.125) ~= bitcast_bf16(int16(x*A + Bc))
# (bf16 = 8-bit exp, 7-bit mantissa -> the int domain is 2^7 per octave)
_LN2 = float(np.log(2.0))
SCH_A = 0.125 * (2**7) / _LN2
SCH_B = 127.0 * (2**7) - 5.5
# k-chunks whose exp runs on VectorE (Schraudolph); rest on ScalarE (exact).
DVE_KCS = frozenset({1, 3, 5, 8, 10, 13, 15})

_CACHE = {}


def _ensure_import():
    try:
        import concourse.bass  # noqa: F401
    except ImportError:
        sys.path.insert(0, "/opt/trn_rl_repo")
        import concourse.bass  # noqa: F401


def build_bass():
    if "nc" in _CACHE:
        return _CACHE["nc"]
    _ensure_import()
    import concourse.mybir as mybir
    import concourse.tile as tile
    from concourse import bacc
    from concourse.masks import make_identity

    f32 = mybir.dt.float32
    f32r = mybir.dt.float32r
    bf16 = mybir.dt.bfloat16
    i16 = mybir.dt.int16
    AF = mybir.ActivationFunctionType
    ALU = mybir.AluOpType

    nc = bacc.Bacc(
        "TRN2",
        target_bir_lowering=False,
        debug=False,
        enable_asserts=False,
        num_devices=NC,
    )
    xT_d = nc.dram_tensor("xT", (D, BS), bf16, kind="ExternalInput").ap()
    wq_d = nc.dram_tensor("wqT", (D, CW), bf16, kind="ExternalInput").ap()
    wk_d = nc.dram_tensor("wkT", (D, CW), bf16, kind="ExternalInput").ap()
    wv_d = nc.dram_tensor("wvT", (D, CW), bf16, kind="ExternalInput").ap()
    bq_d = nc.dram_tensor("bq", (CW, 1), f32, kind="ExternalInput").ap()
    bk_d = nc.dram_tensor("bk", (CW, 1), f32, kind="ExternalInput").ap()
    bv_d = nc.dram_tensor("bv", (CW, 1), f32, kind="ExternalInput").ap()
    mask_d = nc.dram_tensor("maskT", (128, B * KCH), f32, kind="ExternalInput").ap()
    out_d = nc.dram_tensor("h_out", (BS, CW), f32, kind="ExternalOutput").ap()

    with tile.TileContext(nc) as tc:
        with (
            tc.tile_pool(name="qkv", bufs=1) as qkv_pool,
            tc.tile_pool(name="xt", bufs=20) as xt_pool,
            tc.tile_pool(name="wsb", bufs=1) as w_pool,
            tc.tile_pool(name="probs", bufs=6) as probs_pool,
            tc.tile_pool(name="v2", bufs=2) as v2_pool,
            tc.tile_pool(name="hts", bufs=6) as hts_pool,
            tc.tile_pool(name="ho", bufs=4) as ho_pool,
            tc.tile_pool(name="rc", bufs=16) as rc_pool,
            tc.tile_pool(name="cst", bufs=1) as cst_pool,
            tc.tile_pool(name="ps_sc", bufs=2, space="PSUM") as ps_sc,
            tc.tile_pool(name="ps_ht", bufs=2, space="PSUM") as ps_ht,
            tc.tile_pool(name="ps_acc", bufs=1, space="PSUM") as ps_acc,
            tc.tile_pool(name="ps_tr", bufs=1, space="PSUM") as ps_tr,
        ):
            ident = cst_pool.tile([128, 128], f32, tag="ident")
            make_identity(nc, ident)



            wsbs = []
            for name, dram in (("wq", wq_d), ("wk", wk_d), ("wv", wv_d)):
                w_sb = w_pool.tile([128, DCH * CW], bf16, tag=name)
                nc.sync.dma_start(
                    out=w_sb.rearrange("p (c w) -> p c w", c=DCH),
                    in_=dram.rearrange("(c p) w -> p c w", p=128),
                )
                wsbs.append(w_sb)
            bsbs = []
            for name, dram in (("bq", bq_d), ("bk", bk_d), ("bv", bv_d)):
                b_sb = cst_pool.tile([128, 1], f32, tag=name)
                nc.sync.dma_start(out=b_sb, in_=dram)
                bsbs.append(b_sb)
            mask_sb = cst_pool.tile([128, B * KCH], f32, tag="mask")
            nc.sync.dma_start(out=mask_sb, in_=mask_d)

            qt = qkv_pool.tile([128, BS], bf16, tag="qt")
            kt = qkv_pool.tile([128, BS], bf16, tag="kt")
            vt = qkv_pool.tile([128, BS], f32, tag="vt")
            qkv_sb = [qt, kt, vt]

            v2_tiles = {}

            def emit_proj_xts(s_):
                xts = []
                for d in range(DCH):
                    xt_t = xt_pool.tile([128, 512], bf16, tag="xt", name=f"xt{s_}_{d}")
                    nc.sync.dma_start(
                        out=xt_t,
                        in_=xT_d[d * 128 : (d + 1) * 128, s_ * 512 : (s_ + 1) * 512],
                    )
                    xts.append(xt_t)
                return xts

            def emit_proj_piece(s_, pi, xts):
                acc = ps_acc.tile([128, 512], f32, tag="acc", name=f"pj{s_}_{pi}")
                w_sb = wsbs[pi]
                for d in range(DCH):
                    nc.tensor.matmul(
                        acc,
                        w_sb[:, d * CW : (d + 1) * CW],
                        xts[d],
                        start=(d == 0),
                        stop=(d == DCH - 1),
                    )
                dst = qkv_sb[pi][:, s_ * 512 : (s_ + 1) * 512]
                nc.vector.tensor_scalar_add(dst, acc, bsbs[pi])

            def emit_proj_sblock(s_):
                xts = emit_proj_xts(s_)
                for pi in range(3):
                    emit_proj_piece(s_, pi, xts)

            def emit_v2_chunk(b, i):
                # One packed transpose: VT[128(2 heads x 64w), 128t] -> [128t, 128w].
                if (b, 0) not in v2_tiles:
                    v2 = v2_pool.tile([128, KCH * VCOLS], bf16, tag="v2", name=f"v2_{b}")
                    v2_tiles[(b, 0)] = v2
                v2 = v2_tiles[(b, 0)]
                vtr_full = ps_acc.tile([128, 512], f32, tag="acc", name=f"vtr{b}_{i}")
                vtr = vtr_full[:, 0:128]
                nc.tensor.transpose(
                    vtr, vt[:, b * S + i * 128 : b * S + (i + 1) * 128], ident
                )
                mcol = mask_sb[:, b * KCH + i : b * KCH + i + 1]
                ch = v2[:, i * VCOLS : (i + 1) * VCOLS]
                ch2 = ch.rearrange("p (g w) -> p g w", g=2)
                vtr2 = vtr.rearrange("p (g w) -> p g w", g=2)
                nc.vector.tensor_scalar_mul(ch2[:, :, 0:WH], vtr2, mcol)
                nc.vector.tensor_copy(ch[:, WH : WH + 1], mcol)
                nc.vector.tensor_copy(ch[:, VCOLS - 1 : VCOLS], mcol)

            def emit_attention_qb(b, qb, extra=()):
                # `extra`: list of (kg_slot, fn) emitted at the top of that kg
                # iteration — used to interleave next-batch proj/v2 PE work so
                # PSUM-drain latencies hide under attention matmuls.
                extra_by_kg = {}
                for slot, fn in extra:
                    extra_by_kg.setdefault(slot, []).append(fn)
                v2 = v2_tiles[(b, 0)]
                base = b * S
                qs = base + qb * 512
                ht0 = ps_ht.tile([65, 512], f32, tag="ht", name=f"ht0_{b}_{qb}")
                ht1 = ps_ht.tile([65, 512], f32, tag="ht", name=f"ht1_{b}_{qb}")
                def emit_scores_exp(kc):
                    sc = ps_sc.tile(
                        [128, 1024], f32, tag="sc", name=f"sc{b}_{qb}_{kc}"
                    )
                    ks = base + kc * 128
                    nc.tensor.matmul(
                        sc[:, 0:512],
                        kt[0:64, ks : ks + 128],
                        qt[0:64, qs : qs + 512],
                        start=True,
                        stop=True,
                    )
                    nc.tensor.matmul(
                        sc[:, 512:1024],
                        kt[64:128, ks : ks + 128],
                        qt[64:128, qs : qs + 512],
                        start=True,
                        stop=True,
                    )
                    pb = probs_pool.tile(
                        [128, 1024], bf16, tag="pb", name=f"pb{b}_{qb}_{kc}"
                    )
                    if b == B - 1:
                        # tail batch has no proj filler; halve the exp chain
                        # latency by splitting each tile over both engines
                        nc.scalar.activation(
                            pb[:, 0:512], sc[:, 0:512], AF.Exp, scale=0.125
                        )
                        nc.vector.tensor_scalar(
                            pb[:, 512:1024].bitcast(i16),
                            sc[:, 512:1024],
                            SCH_A,
                            SCH_B,
                            ALU.mult,
                            ALU.add,
                        )
                    elif kc in DVE_KCS:
                        nc.vector.tensor_scalar(
                            pb.bitcast(i16), sc, SCH_A, SCH_B, ALU.mult, ALU.add
                        )
                    else:
                        nc.scalar.activation(pb, sc, AF.Exp, scale=0.125)
                    return pb

                def emit_pv(kc, pb):
                    c0 = kc * VCOLS
                    nc.tensor.matmul(
                        ht0,
                        v2[:, c0 : c0 + WH + 1],
                        pb[:, 0:512],
                        start=(kc == 0),
                        stop=(kc == KCH - 1),
                        skip_group_check=True,
                    )
                    nc.tensor.matmul(
                        ht1,
                        v2[:, c0 + WH + 1 : c0 + VCOLS],
                        pb[:, 512:1024],
                        start=(kc == 0),
                        stop=(kc == KCH - 1),
                        skip_group_check=True,
                    )

                # Software pipeline: scores/exp run one k-chunk ahead of pv so
                # the PE always has an independent matmul pair while exp runs.
                prev_pb = None
                for kc in range(KCH):
                    if kc % 2 == 0:
                        for fn in extra_by_kg.get(kc // 2, ()):
                            fn()
                    pb = emit_scores_exp(kc)
                    if prev_pb is not None:
                        emit_pv(kc - 1, prev_pb)
                    prev_pb = pb
                emit_pv(KCH - 1, prev_pb)
                hts_t = []
                for hh, ht in ((0, ht0), (1, ht1)):
                    hts = hts_pool.tile(
                        [65, 512], f32, tag="hts", name=f"hts{b}_{qb}_{hh}"
                    )
                    if hh == 0:
                        nc.scalar.copy(hts, ht)
                    else:
                        nc.vector.tensor_copy(hts, ht)
                    hts_t.append(hts)

                # Output epilogue, split into pieces so the two heads can be
                # interleaved (one head's DVE work hides under the other's PE
                # transpose). One [128,288] tr tile per qb = ONE PSUM bank
                # holding four independent 72-col slices (2 per head).
                state = {}

                def epi_piece(hh, t):
                    if "tr" not in state:
                        state["tr"] = ps_tr.tile(
                            [128, 288], f32, tag="tr", name=f"tr{b}_{qb}"
                        )
                        state["ho"] = [
                            ho_pool.tile(
                                [128, 256], f32, tag="ho", name=f"ho{b}_{qb}_{h}"
                            )
                            for h in range(2)
                        ]
                    trt, ho = state["tr"], state["ho"][hh]
                    hts = hts_t[hh]
                    off = hh * 144 + (t % 2) * 72
                    tr2 = trt[:, off : off + 72]
                    nc.tensor.transpose(
                        tr2[:, 0:65],
                        hts[:, t * 128 : (t + 1) * 128],
                        ident[0:65, 0:65],
                    )
                    rc = rc_pool.tile(
                        [128, 1], f32, tag="rc", name=f"rc{b}_{qb}_{hh}_{t}"
                    )
                    nc.vector.reciprocal(rc, tr2[:, 64:65])
                    nc.vector.tensor_scalar_mul(
                        ho[:, t * 64 : (t + 1) * 64], tr2[:, 0:64], rc
                    )
                    if t == 3:
                        hp = hh * WH
                        dst = out_d[qs : qs + 512, hp : hp + 64].rearrange(
                            "(t p) w -> p t w", p=128
                        )
                        nc.gpsimd.dma_start(
                            out=dst, in_=ho.rearrange("p (t w) -> p t w", t=4)
                        )

                return [
                    (lambda hh=hh, t=t: epi_piece(hh, t))
                    for t in range(4)
                    for hh in range(2)
                ]

            # ---- emission: proj/v2 of batch b+1 interleaved with attention(b) ----
            for s in range(4):
                emit_proj_sblock(s)
                for c in range(4 * s, 4 * s + 4):
                    emit_v2_chunk(0, c)
            pending_epis = []
            for b in range(B):
                for qb in range(QB):
                    if b + 1 < B:
                        s_ = 4 * (b + 1) + qb
                        emit_proj_sblock(s_)
                        for c in range(4 * qb, 4 * qb + 4):
                            emit_v2_chunk(b + 1, c)
                    # defer the previous qb's output epilogues into this qb's
                    # kc loop: the PE transposes then overlap attention instead
                    # of stalling on the cross-engine hts-copy latency
                    extra = [(i, e) for i, e in enumerate(pending_epis)]
                    pending_epis = emit_attention_qb(b, qb, extra)
            for e in pending_epis:
                e()

    nc.compile()
    _CACHE["nc"] = nc
    return nc


def make_in_maps(x, mask, Wq, bq, Wk, bk, Wv, bv):
    import ml_dtypes

    bf = ml_dtypes.bfloat16
    x = np.asarray(x, dtype=np.float32)
    xT = np.ascontiguousarray(x.reshape(BS, D).T.astype(bf))
    maskT = np.ascontiguousarray(
        np.asarray(mask, dtype=np.float32)
        .reshape(B, KCH, 128)
        .transpose(2, 0, 1)
        .reshape(128, B * KCH)
    )
    in_maps = []
    for c in range(NC):
        cols = slice(c * CW, (c + 1) * CW)
        in_maps.append(
            {
                "xT": xT,
                "wqT": np.ascontiguousarray(np.asarray(Wq, np.float32)[cols, :].T.astype(bf)),
                "wkT": np.ascontiguousarray(np.asarray(Wk, np.float32)[cols, :].T.astype(bf)),
                "wvT": np.ascontiguousarray(np.asarray(Wv, np.float32)[cols, :].T.astype(bf)),
                "bq": np.ascontiguousarray(np.asarray(bq, np.float32)[cols, None]),
                "bk": np.ascontiguousarray(np.asarray(bk, np.float32)[cols, None]),
                "bv": np.ascontiguousarray(np.asarray(bv, np.float32)[cols, None]),
                "maskT": maskT,
            }
        )
    return in_maps


def assemble(results):
    out = np.empty((BS, D), dtype=np.float32)
    for c in range(NC):
        out[:, c * CW : (c + 1) * CW] = results[c]["h_out"]
    return out.reshape(B, S, D)


def kernel(x, mask, Wq, bq, Wk, bk, Wv, bv, **run_kwargs):
    _ensure_import()
    from concourse.bass_utils import run_bass_kernel_spmd

    nc = build_bass()
    in_maps = make_in_maps(x, mask, Wq, bq, Wk, bk, Wv, bv)
    res = run_bass_kernel_spmd(nc, in_maps, core_ids=list(range(NC)), **run_kwargs)
    _CACHE["last_results"] = res
    return assemble(res.results)


# revision 63
# speedup vs baseline: 1.0803x; 1.0358x over previous
# Multi-headed self-attention (B=4, S=2048, D=1024, H=16) on 8 TRN2 NeuronCores.
#
# Sharding: tensor-parallel over heads. Core c computes heads 2c, 2c+1 (=128
# output columns) for all batches. Host pre-transposes x -> xT [D, B*S] and the
# per-core weight slices -> [D, 128] so every matmul contracts over the
# partition dimension. Host gathers the 8 [B*S, 128] outputs into (B,S,D).
#
# Per-core dataflow (bf16 PE operands, fp32 PSUM accumulation):
#   1. Projections (bf16 x, bf16 W): QT/KT [128(2 heads x 64), 8192] bf16 and
#      VT f32, accumulated over 8 d-chunks in PSUM; bias added during the
#      PSUM->SBUF move (DVE per-partition scalar add).
#   2. V2 prep: one packed PE transpose per 128-t chunk turns VT[128(2 heads),
#      128t] into [128t, 128w]; DVE applies the key mask and appends a mask
#      column per head -> v2 chunk layout [V_h0(64)|m|V_h1(64)|m] (130 cols,
#      bf16).
#   3. Attention per (batch, q-block): per k-chunk ONE [128, 1024] PSUM tile
#      holds both heads' scoresT [128 k, 512 q]; the two K=64 bf16 matmuls
#      sit on disjoint PE row halves (partitions 0:64 / 64:128) and their
#      moving streams overlap (~1.4 cols/cycle aggregate, beating the 1
#      col/cycle single-matmul floor). exp alternates engines per k-chunk:
#      ScalarE exact Exp (fused 1/8 scale) for 9/16, VectorE Schraudolph
#      bit-trick exp (one tensor_scalar: bf16 bits = int16(x*A+B), ~1.8% rel
#      err, softmax-safe since the denominator uses the same approximation)
#      for 7/16 — exactly one VectorE chunk per scores-PSUM pair so the two
#      engines always run concurrently. No row-max subtraction (scores std
#      ~0.4, exp is safe, softmax is shift-invariant). Scores/exp run one
#      k-chunk ahead of the PV matmuls (software pipeline). PV matmuls use
#      the 65-col bf16 stationary [V|mask] so the accumulation yields
#      unnormalized h^T plus the softmax denominator. PE-transpose h''^T back
#      to [q, 65], DVE reciprocal of column 64, per-partition scalar
#      multiply, DMA out. hts PSUM->SBUF copies split ScalarE/VectorE.
#   The 0/1 mask is exact this way: reference's exp(-10000) == 0.0 in fp32.
#   Emission interleaves proj/v2-prep of batch b+1 with attention of batch b
#   to keep the PE dense; each q-block's output epilogue (transpose/normalize/
#   DMA) is deferred into the NEXT q-block's kc loop so the PE transposes
#   overlap attention instead of stalling on cross-engine copy latency.
#   bf16 quantization of x/W/Q/K/V/probs plus the Schraudolph share puts the
#   end-to-end rel error at ~8e-3 (gate 2e-2).

import sys

import numpy as np

B, S, D, H = 4, 2048, 1024, 16
NC = 8
HPC = H // NC  # heads per core = 2
WH = D // H  # head width = 64
CW = HPC * WH  # per-core output width = 128
BS = B * S  # 8192
DCH = D // 128  # d chunks = 8
QB = S // 512  # q blocks per batch = 4
KCH = S // 128  # k chunks per batch = 16
VCOLS = 2 * (WH + 1)  # v2 chunk cols = 130

# Schraudolph exp in bf16: exp(x*0.125) ~= bitcast_bf16(int16(x*A + Bc))
# (bf16 = 8-bit exp, 7-bit mantissa -> the int domain is 2^7 per octave)
_LN2 = float(np.log(2.0))
SCH_A = 0.125 * (2**7) / _LN2
SCH_B = 127.0 * (2**7) - 5.5
# k-chunks whose exp runs on VectorE (Schraudolph); rest on ScalarE (exact).
DVE_KCS = frozenset({1, 3, 5, 8, 10, 13, 15})

_CACHE = {}


def _ensure_import():
    try:
        import concourse.bass  # noqa: F401
    except ImportError:
        sys.path.insert(0, "/opt/trn_rl_repo")
        import concourse.bass  # noqa: F401


def build_bass():
    if "nc" in _CACHE:
        return _CACHE["nc"]
    _ensure_import()
    import concourse.mybir as mybir
    import concourse.tile as tile
    from concourse import bacc
    from concourse.masks import make_identity

    f32 = mybir.dt.float32
    f32r = mybir.dt.float32r
    bf16 = mybir.dt.bfloat16
    i16 = mybir.dt.int16
    AF = mybir.ActivationFunctionType
    ALU = mybir.AluOpType

    nc = bacc.Bacc(
        "TRN2",
        target_bir_lowering=False,
        debug=False,
        enable_asserts=False,
        num_devices=NC,
    )
    xT_d = nc.dram_tensor("xT", (D, BS), bf16, kind="ExternalInput").ap()
    wq_d = nc.dram_tensor("wqT", (D, CW), bf16, kind="ExternalInput").ap()
    wk_d = nc.dram_tensor("wkT", (D, CW), bf16, kind="ExternalInput").ap()
    wv_d = nc.dram_tensor("wvT", (D, CW), bf16, kind="ExternalInput").ap()
    bq_d = nc.dram_tensor("bq", (CW, 1), f32, kind="ExternalInput").ap()
    bk_d = nc.dram_tensor("bk", (CW, 1), f32, kind="ExternalInput").ap()
    bv_d = nc.dram_tensor("bv", (CW, 1), f32, kind="ExternalInput").ap()
    mask_d = nc.dram_tensor("maskT", (128, B * KCH), f32, kind="ExternalInput").ap()
    out_d = nc.dram_tensor("h_out", (BS, CW), f32, kind="ExternalOutput").ap()

    with tile.TileContext(nc) as tc:
        with (
            tc.tile_pool(name="qkv", bufs=1) as qkv_pool,
            tc.tile_pool(name="xt", bufs=20) as xt_pool,
            tc.tile_pool(name="wsb", bufs=1) as w_pool,
            tc.tile_pool(name="probs", bufs=6) as probs_pool,
            tc.tile_pool(name="v2", bufs=2) as v2_pool,
            tc.tile_pool(name="hts", bufs=6) as hts_pool,
            tc.tile_pool(name="ho", bufs=4) as ho_pool,
            tc.tile_pool(name="rc", bufs=16) as rc_pool,
            tc.tile_pool(name="cst", bufs=1) as cst_pool,
            tc.tile_pool(name="ps_sc", bufs=2, space="PSUM") as ps_sc,
            tc.tile_pool(name="ps_ht", bufs=2, space="PSUM") as ps_ht,
            tc.tile_pool(name="ps_acc", bufs=1, space="PSUM") as ps_acc,
            tc.tile_pool(name="ps_tr", bufs=1, space="PSUM") as ps_tr,
        ):
            ident = cst_pool.tile([128, 128], f32, tag="ident")
            make_identity(nc, ident)



            wsbs = []
            for name, dram in (("wq", wq_d), ("wk", wk_d), ("wv", wv_d)):
                w_sb = w_pool.tile([128, DCH * CW], bf16, tag=name)
                nc.sync.dma_start(
                    out=w_sb.rearrange("p (c w) -> p c w", c=DCH),
                    in_=dram.rearrange("(c p) w -> p c w", p=128),
                )
                wsbs.append(w_sb)
            bsbs = []
            for name, dram in (("bq", bq_d), ("bk", bk_d), ("bv", bv_d)):
                b_sb = cst_pool.tile([128, 1], f32, tag=name)
                nc.sync.dma_start(out=b_sb, in_=dram)
                bsbs.append(b_sb)
            mask_sb = cst_pool.tile([128, B * KCH], f32, tag="mask")
            nc.sync.dma_start(out=mask_sb, in_=mask_d)

            qt = qkv_pool.tile([128, BS], bf16, tag="qt")
            kt = qkv_pool.tile([128, BS], bf16, tag="kt")
            vt = qkv_pool.tile([128, BS], f32, tag="vt")
            qkv_sb = [qt, kt, vt]

            v2_tiles = {}

            def emit_proj_xts(s_):
                xts = []
                for d in range(DCH):
                    xt_t = xt_pool.tile([128, 512], bf16, tag="xt", name=f"xt{s_}_{d}")
                    nc.sync.dma_start(
                        out=xt_t,
                        in_=xT_d[d * 128 : (d + 1) * 128, s_ * 512 : (s_ + 1) * 512],
                    )
                    xts.append(xt_t)
                return xts

            def emit_proj_piece(s_, pi, xts, prologue=False):
                if prologue:
                    # attention hasn't started: borrow the idle sc-pool banks
                    # so prologue proj groups double-buffer instead of
                    # stalling on the single acc bank
                    acc = ps_sc.tile(
                        [128, 1024], f32, tag="sc", name=f"pjp{s_}_{pi}"
                    )[:, 0:512]
                else:
                    acc = ps_acc.tile([128, 512], f32, tag="acc", name=f"pj{s_}_{pi}")
                w_sb = wsbs[pi]
                for d in range(DCH):
                    nc.tensor.matmul(
                        acc,
                        w_sb[:, d * CW : (d + 1) * CW],
                        xts[d],
                        start=(d == 0),
                        stop=(d == DCH - 1),
                    )
                dst = qkv_sb[pi][:, s_ * 512 : (s_ + 1) * 512]
                nc.vector.tensor_scalar_add(dst, acc, bsbs[pi])

            def emit_proj_sblock(s_, prologue=False):
                xts = emit_proj_xts(s_)
                for pi in range(3):
                    emit_proj_piece(s_, pi, xts, prologue=prologue)

            def emit_v2_chunk(b, i):
                # One packed transpose: VT[128(2 heads x 64w), 128t] -> [128t, 128w].
                if (b, 0) not in v2_tiles:
                    v2 = v2_pool.tile([128, KCH * VCOLS], bf16, tag="v2", name=f"v2_{b}")
                    v2_tiles[(b, 0)] = v2
                v2 = v2_tiles[(b, 0)]
                vtr_full = ps_acc.tile([128, 512], f32, tag="acc", name=f"vtr{b}_{i}")
                vtr = vtr_full[:, 0:128]
                nc.tensor.transpose(
                    vtr, vt[:, b * S + i * 128 : b * S + (i + 1) * 128], ident
                )
                mcol = mask_sb[:, b * KCH + i : b * KCH + i + 1]
                ch = v2[:, i * VCOLS : (i + 1) * VCOLS]
                ch2 = ch.rearrange("p (g w) -> p g w", g=2)
                vtr2 = vtr.rearrange("p (g w) -> p g w", g=2)
                nc.vector.tensor_scalar_mul(ch2[:, :, 0:WH], vtr2, mcol)
                nc.vector.tensor_copy(ch[:, WH : WH + 1], mcol)
                nc.vector.tensor_copy(ch[:, VCOLS - 1 : VCOLS], mcol)

            def emit_attention_qb(b, qb, extra=()):
                # `extra`: list of (kg_slot, fn) emitted at the top of that kg
                # iteration — used to interleave next-batch proj/v2 PE work so
                # PSUM-drain latencies hide under attention matmuls.
                extra_by_kg = {}
                for slot, fn in extra:
                    extra_by_kg.setdefault(slot, []).append(fn)
                v2 = v2_tiles[(b, 0)]
                base = b * S
                qs = base + qb * 512
                ht0 = ps_ht.tile([65, 512], f32, tag="ht", name=f"ht0_{b}_{qb}")
                ht1 = ps_ht.tile([65, 512], f32, tag="ht", name=f"ht1_{b}_{qb}")
                def emit_scores_exp(kc):
                    sc = ps_sc.tile(
                        [128, 1024], f32, tag="sc", name=f"sc{b}_{qb}_{kc}"
                    )
                    ks = base + kc * 128
                    nc.tensor.matmul(
                        sc[:, 0:512],
                        kt[0:64, ks : ks + 128],
                        qt[0:64, qs : qs + 512],
                        start=True,
                        stop=True,
                    )
                    nc.tensor.matmul(
                        sc[:, 512:1024],
                        kt[64:128, ks : ks + 128],
                        qt[64:128, qs : qs + 512],
                        start=True,
                        stop=True,
                    )
                    pb = probs_pool.tile(
                        [128, 1024], bf16, tag="pb", name=f"pb{b}_{qb}_{kc}"
                    )
                    if b == B - 1:
                        # tail batch has no proj filler; halve the exp chain
                        # latency by splitting each tile over both engines
                        nc.scalar.activation(
                            pb[:, 0:512], sc[:, 0:512], AF.Exp, scale=0.125
                        )
                        nc.vector.tensor_scalar(
                            pb[:, 512:1024].bitcast(i16),
                            sc[:, 512:1024],
                            SCH_A,
                            SCH_B,
                            ALU.mult,
                            ALU.add,
                        )
                    elif kc in DVE_KCS:
                        nc.vector.tensor_scalar(
                            pb.bitcast(i16), sc, SCH_A, SCH_B, ALU.mult, ALU.add
                        )
                    else:
                        nc.scalar.activation(pb, sc, AF.Exp, scale=0.125)
                    return pb

                def emit_pv(kc, pb):
                    c0 = kc * VCOLS
                    nc.tensor.matmul(
                        ht0,
                        v2[:, c0 : c0 + WH + 1],
                        pb[:, 0:512],
                        start=(kc == 0),
                        stop=(kc == KCH - 1),
                        skip_group_check=True,
                    )
                    nc.tensor.matmul(
                        ht1,
                        v2[:, c0 + WH + 1 : c0 + VCOLS],
                        pb[:, 512:1024],
                        start=(kc == 0),
                        stop=(kc == KCH - 1),
                        skip_group_check=True,
                    )

                # Software pipeline: scores/exp run one k-chunk ahead of pv so
                # the PE always has an independent matmul pair while exp runs.
                prev_pb = None
                for kc in range(KCH):
                    if kc % 2 == 0:
                        for fn in extra_by_kg.get(kc // 2, ()):
                            fn()
                    pb = emit_scores_exp(kc)
                    if prev_pb is not None:
                        emit_pv(kc - 1, prev_pb)
                    prev_pb = pb
                emit_pv(KCH - 1, prev_pb)
                hts_t = []
                for hh, ht in ((0, ht0), (1, ht1)):
                    hts = hts_pool.tile(
                        [65, 512], f32, tag="hts", name=f"hts{b}_{qb}_{hh}"
                    )
                    if hh == 0:
                        nc.scalar.copy(hts, ht)
                    else:
                        nc.vector.tensor_copy(hts, ht)
                    hts_t.append(hts)

                # Output epilogue, split into pieces so the two heads can be
                # interleaved (one head's DVE work hides under the other's PE
                # transpose). One [128,288] tr tile per qb = ONE PSUM bank
                # holding four independent 72-col slices (2 per head).
                state = {}

                def epi_piece(hh, t, final=False):
                    if "ho" not in state:
                        state["ho"] = [
                            ho_pool.tile(
                                [128, 256], f32, tag="ho", name=f"ho{b}_{qb}_{h}"
                            )
                            for h in range(2)
                        ]
                    if final:
                        # very last epilogue: attention is over, borrow idle
                        # sc-pool banks for 4 independent slices per head so
                        # the transpose->reciprocal->mul chains fully overlap
                        key = f"trf{hh}"
                        if key not in state:
                            state[key] = ps_sc.tile(
                                [128, 1024], f32, tag="sc", name=f"trf{b}_{qb}_{hh}"
                            )
                        tr2 = state[key][:, t * 72 : t * 72 + 72]
                    else:
                        if "tr" not in state:
                            state["tr"] = ps_tr.tile(
                                [128, 288], f32, tag="tr", name=f"tr{b}_{qb}"
                            )
                        off = hh * 144 + (t % 2) * 72
                        tr2 = state["tr"][:, off : off + 72]
                    ho = state["ho"][hh]
                    hts = hts_t[hh]
                    nc.tensor.transpose(
                        tr2[:, 0:65],
                        hts[:, t * 128 : (t + 1) * 128],
                        ident[0:65, 0:65],
                    )
                    rc = rc_pool.tile(
                        [128, 1], f32, tag="rc", name=f"rc{b}_{qb}_{hh}_{t}"
                    )
                    nc.vector.reciprocal(rc, tr2[:, 64:65])
                    nc.vector.tensor_scalar_mul(
                        ho[:, t * 64 : (t + 1) * 64], tr2[:, 0:64], rc
                    )
                    if t == 3:
                        hp = hh * WH
                        dst = out_d[qs : qs + 512, hp : hp + 64].rearrange(
                            "(t p) w -> p t w", p=128
                        )
                        nc.gpsimd.dma_start(
                            out=dst, in_=ho.rearrange("p (t w) -> p t w", t=4)
                        )

                return [
                    (lambda hh=hh, t=t, **kw: epi_piece(hh, t, **kw))
                    for t in range(4)
                    for hh in range(2)
                ]

            # ---- emission: proj/v2 of batch b+1 interleaved with attention(b) ----
            for s in range(4):
                emit_proj_sblock(s, prologue=True)
                for c in range(4 * s, 4 * s + 4):
                    emit_v2_chunk(0, c)
            pending_epis = []
            for b in range(B):
                for qb in range(QB):
                    if b + 1 < B:
                        s_ = 4 * (b + 1) + qb
                        emit_proj_sblock(s_)
                        for c in range(4 * qb, 4 * qb + 4):
                            emit_v2_chunk(b + 1, c)
                    # defer the previous qb's output epilogues into this qb's
                    # kc loop: the PE transposes then overlap attention instead
                    # of stalling on the cross-engine hts-copy latency
                    extra = [(i, e) for i, e in enumerate(pending_epis)]
                    pending_epis = emit_attention_qb(b, qb, extra)
            for e in pending_epis:
                e(final=True)

    nc.compile()
    _CACHE["nc"] = nc
    return nc


def make_in_maps(x, mask, Wq, bq, Wk, bk, Wv, bv):
    import ml_dtypes

    bf = ml_dtypes.bfloat16
    x = np.asarray(x, dtype=np.float32)
    xT = np.ascontiguousarray(x.reshape(BS, D).T.astype(bf))
    maskT = np.ascontiguousarray(
        np.asarray(mask, dtype=np.float32)
        .reshape(B, KCH, 128)
        .transpose(2, 0, 1)
        .reshape(128, B * KCH)
    )
    in_maps = []
    for c in range(NC):
        cols = slice(c * CW, (c + 1) * CW)
        in_maps.append(
            {
                "xT": xT,
                "wqT": np.ascontiguousarray(np.asarray(Wq, np.float32)[cols, :].T.astype(bf)),
                "wkT": np.ascontiguousarray(np.asarray(Wk, np.float32)[cols, :].T.astype(bf)),
                "wvT": np.ascontiguousarray(np.asarray(Wv, np.float32)[cols, :].T.astype(bf)),
                "bq": np.ascontiguousarray(np.asarray(bq, np.float32)[cols, None]),
                "bk": np.ascontiguousarray(np.asarray(bk, np.float32)[cols, None]),
                "bv": np.ascontiguousarray(np.asarray(bv, np.float32)[cols, None]),
                "maskT": maskT,
            }
        )
    return in_maps


def assemble(results):
    out = np.empty((BS, D), dtype=np.float32)
    for c in range(NC):
        out[:, c * CW : (c + 1) * CW] = results[c]["h_out"]
    return out.reshape(B, S, D)


def kernel(x, mask, Wq, bq, Wk, bk, Wv, bv, **run_kwargs):
    _ensure_import()
    from concourse.bass_utils import run_bass_kernel_spmd

    nc = build_bass()
    in_maps = make_in_maps(x, mask, Wq, bq, Wk, bk, Wv, bv)
    res = run_bass_kernel_spmd(nc, in_maps, core_ids=list(range(NC)), **run_kwargs)
    _CACHE["last_results"] = res
    return assemble(res.results)


# revision 64
# speedup vs baseline: 1.0804x; 1.0001x over previous
# Multi-headed self-attention (B=4, S=2048, D=1024, H=16) on 8 TRN2 NeuronCores.
#
# Sharding: tensor-parallel over heads. Core c computes heads 2c, 2c+1 (=128
# output columns) for all batches. Host pre-transposes x -> xT [D, B*S] and the
# per-core weight slices -> [D, 128] so every matmul contracts over the
# partition dimension. Host gathers the 8 [B*S, 128] outputs into (B,S,D).
#
# Per-core dataflow (bf16 PE operands, fp32 PSUM accumulation):
#   1. Projections (bf16 x, bf16 W): QT/KT [128(2 heads x 64), 8192] bf16 and
#      VT f32, accumulated over 8 d-chunks in PSUM; bias added during the
#      PSUM->SBUF move (DVE per-partition scalar add).
#   2. V2 prep: one packed PE transpose per 128-t chunk turns VT[128(2 heads),
#      128t] into [128t, 128w]; DVE applies the key mask and appends a mask
#      column per head -> v2 chunk layout [V_h0(64)|m|V_h1(64)|m] (130 cols,
#      bf16).
#   3. Attention per (batch, q-block): per k-chunk ONE [128, 1024] PSUM tile
#      holds both heads' scoresT [128 k, 512 q]; the two K=64 bf16 matmuls
#      sit on disjoint PE row halves (partitions 0:64 / 64:128) and their
#      moving streams overlap (~1.4 cols/cycle aggregate, beating the 1
#      col/cycle single-matmul floor). exp alternates engines per k-chunk:
#      ScalarE exact Exp (fused 1/8 scale) for 9/16, VectorE Schraudolph
#      bit-trick exp (one tensor_scalar: bf16 bits = int16(x*A+B), ~1.8% rel
#      err, softmax-safe since the denominator uses the same approximation)
#      for 7/16 — exactly one VectorE chunk per scores-PSUM pair so the two
#      engines always run concurrently. No row-max subtraction (scores std
#      ~0.4, exp is safe, softmax is shift-invariant). Scores/exp run one
#      k-chunk ahead of the PV matmuls (software pipeline). PV matmuls use
#      the 65-col bf16 stationary [V|mask] so the accumulation yields
#      unnormalized h^T plus the softmax denominator. PE-transpose h''^T back
#      to [q, 65], DVE reciprocal of column 64, per-partition scalar
#      multiply, DMA out. hts PSUM->SBUF copies split ScalarE/VectorE.
#   The 0/1 mask is exact this way: reference's exp(-10000) == 0.0 in fp32.
#   Emission interleaves proj/v2-prep of batch b+1 with attention of batch b
#   to keep the PE dense; each q-block's output epilogue (transpose/normalize/
#   DMA) is deferred into the NEXT q-block's kc loop so the PE transposes
#   overlap attention instead of stalling on cross-engine copy latency.
#   bf16 quantization of x/W/Q/K/V/probs plus the Schraudolph share puts the
#   end-to-end rel error at ~8e-3 (gate 2e-2).

import sys

import numpy as np

B, S, D, H = 4, 2048, 1024, 16
NC = 8
HPC = H // NC  # heads per core = 2
WH = D // H  # head width = 64
CW = HPC * WH  # per-core output width = 128
BS = B * S  # 8192
DCH = D // 128  # d chunks = 8
QB = S // 512  # q blocks per batch = 4
KCH = S // 128  # k chunks per batch = 16
VCOLS = 2 * (WH + 1)  # v2 chunk cols = 130

# Schraudolph exp in bf16: exp(x*0.125) ~= bitcast_bf16(int16(x*A + Bc))
# (bf16 = 8-bit exp, 7-bit mantissa -> the int domain is 2^7 per octave)
_LN2 = float(np.log(2.0))
SCH_A = 0.125 * (2**7) / _LN2
SCH_B = 127.0 * (2**7) - 5.5
# k-chunks whose exp runs on VectorE (Schraudolph); rest on ScalarE (exact).
DVE_KCS = frozenset({1, 3, 5, 8, 10, 13, 15})

_CACHE = {}


def _ensure_import():
    try:
        import concourse.bass  # noqa: F401
    except ImportError:
        sys.path.insert(0, "/opt/trn_rl_repo")
        import concourse.bass  # noqa: F401


def build_bass():
    if "nc" in _CACHE:
        return _CACHE["nc"]
    _ensure_import()
    import concourse.mybir as mybir
    import concourse.tile as tile
    from concourse import bacc
    from concourse.masks import make_identity

    f32 = mybir.dt.float32
    f32r = mybir.dt.float32r
    bf16 = mybir.dt.bfloat16
    i16 = mybir.dt.int16
    AF = mybir.ActivationFunctionType
    ALU = mybir.AluOpType

    nc = bacc.Bacc(
        "TRN2",
        target_bir_lowering=False,
        debug=False,
        enable_asserts=False,
        num_devices=NC,
    )
    xT_d = nc.dram_tensor("xT", (D, BS), bf16, kind="ExternalInput").ap()
    wq_d = nc.dram_tensor("wqT", (D, CW), bf16, kind="ExternalInput").ap()
    wk_d = nc.dram_tensor("wkT", (D, CW), bf16, kind="ExternalInput").ap()
    wv_d = nc.dram_tensor("wvT", (D, CW), bf16, kind="ExternalInput").ap()
    bq_d = nc.dram_tensor("bq", (CW, 1), f32, kind="ExternalInput").ap()
    bk_d = nc.dram_tensor("bk", (CW, 1), f32, kind="ExternalInput").ap()
    bv_d = nc.dram_tensor("bv", (CW, 1), f32, kind="ExternalInput").ap()
    mask_d = nc.dram_tensor("maskT", (128, B * KCH), f32, kind="ExternalInput").ap()
    out_d = nc.dram_tensor("h_out", (BS, CW), f32, kind="ExternalOutput").ap()

    with tile.TileContext(nc) as tc:
        with (
            tc.tile_pool(name="qkv", bufs=1) as qkv_pool,
            tc.tile_pool(name="xt", bufs=20) as xt_pool,
            tc.tile_pool(name="wsb", bufs=1) as w_pool,
            tc.tile_pool(name="probs", bufs=6) as probs_pool,
            tc.tile_pool(name="v2", bufs=2) as v2_pool,
            tc.tile_pool(name="hts", bufs=6) as hts_pool,
            tc.tile_pool(name="ho", bufs=4) as ho_pool,
            tc.tile_pool(name="rc", bufs=16) as rc_pool,
            tc.tile_pool(name="cst", bufs=1) as cst_pool,
            tc.tile_pool(name="ps_sc", bufs=2, space="PSUM") as ps_sc,
            tc.tile_pool(name="ps_ht", bufs=2, space="PSUM") as ps_ht,
            tc.tile_pool(name="ps_acc", bufs=1, space="PSUM") as ps_acc,
            tc.tile_pool(name="ps_tr", bufs=1, space="PSUM") as ps_tr,
        ):
            ident = cst_pool.tile([128, 128], f32, tag="ident")
            make_identity(nc, ident)



            wsbs = []
            for name, dram in (("wq", wq_d), ("wk", wk_d), ("wv", wv_d)):
                w_sb = w_pool.tile([128, DCH * CW], bf16, tag=name)
                nc.sync.dma_start(
                    out=w_sb.rearrange("p (c w) -> p c w", c=DCH),
                    in_=dram.rearrange("(c p) w -> p c w", p=128),
                )
                wsbs.append(w_sb)
            bsbs = []
            for name, dram in (("bq", bq_d), ("bk", bk_d), ("bv", bv_d)):
                b_sb = cst_pool.tile([128, 1], f32, tag=name)
                nc.sync.dma_start(out=b_sb, in_=dram)
                bsbs.append(b_sb)
            mask_sb = cst_pool.tile([128, B * KCH], f32, tag="mask")
            nc.sync.dma_start(out=mask_sb, in_=mask_d)

            qt = qkv_pool.tile([128, BS], bf16, tag="qt")
            kt = qkv_pool.tile([128, BS], bf16, tag="kt")
            vt = qkv_pool.tile([128, BS], f32, tag="vt")
            qkv_sb = [qt, kt, vt]

            v2_tiles = {}

            def emit_proj_xts(s_):
                xts = []
                for d in range(DCH):
                    xt_t = xt_pool.tile([128, 512], bf16, tag="xt", name=f"xt{s_}_{d}")
                    nc.sync.dma_start(
                        out=xt_t,
                        in_=xT_d[d * 128 : (d + 1) * 128, s_ * 512 : (s_ + 1) * 512],
                    )
                    xts.append(xt_t)
                return xts

            def emit_proj_piece(s_, pi, xts, prologue=False):
                if prologue:
                    # attention hasn't started: borrow the idle sc-pool banks
                    # so prologue proj groups double-buffer instead of
                    # stalling on the single acc bank
                    acc = ps_sc.tile(
                        [128, 1024], f32, tag="sc", name=f"pjp{s_}_{pi}"
                    )[:, 0:512]
                else:
                    acc = ps_acc.tile([128, 512], f32, tag="acc", name=f"pj{s_}_{pi}")
                w_sb = wsbs[pi]
                for d in range(DCH):
                    nc.tensor.matmul(
                        acc,
                        w_sb[:, d * CW : (d + 1) * CW],
                        xts[d],
                        start=(d == 0),
                        stop=(d == DCH - 1),
                    )
                dst = qkv_sb[pi][:, s_ * 512 : (s_ + 1) * 512]
                nc.vector.tensor_scalar_add(dst, acc, bsbs[pi])

            def emit_proj_sblock(s_, prologue=False):
                xts = emit_proj_xts(s_)
                for pi in range(3):
                    emit_proj_piece(s_, pi, xts, prologue=prologue)

            def emit_v2_chunk(b, i):
                # One packed transpose: VT[128(2 heads x 64w), 128t] -> [128t, 128w].
                if (b, 0) not in v2_tiles:
                    v2 = v2_pool.tile([128, KCH * VCOLS], bf16, tag="v2", name=f"v2_{b}")
                    v2_tiles[(b, 0)] = v2
                v2 = v2_tiles[(b, 0)]
                vtr_full = ps_acc.tile([128, 512], f32, tag="acc", name=f"vtr{b}_{i}")
                vtr = vtr_full[:, 0:128]
                nc.tensor.transpose(
                    vtr, vt[:, b * S + i * 128 : b * S + (i + 1) * 128], ident
                )
                mcol = mask_sb[:, b * KCH + i : b * KCH + i + 1]
                ch = v2[:, i * VCOLS : (i + 1) * VCOLS]
                ch2 = ch.rearrange("p (g w) -> p g w", g=2)
                vtr2 = vtr.rearrange("p (g w) -> p g w", g=2)
                nc.vector.tensor_scalar_mul(ch2[:, :, 0:WH], vtr2, mcol)
                nc.vector.tensor_copy(ch[:, WH : WH + 1], mcol)
                nc.vector.tensor_copy(ch[:, VCOLS - 1 : VCOLS], mcol)

            def emit_attention_qb(b, qb, extra=()):
                # `extra`: list of (kg_slot, fn) emitted at the top of that kg
                # iteration — used to interleave next-batch proj/v2 PE work so
                # PSUM-drain latencies hide under attention matmuls.
                extra_by_kg = {}
                for slot, fn in extra:
                    extra_by_kg.setdefault(slot, []).append(fn)
                v2 = v2_tiles[(b, 0)]
                base = b * S
                qs = base + qb * 512
                ht0 = ps_ht.tile([65, 512], f32, tag="ht", name=f"ht0_{b}_{qb}")
                ht1 = ps_ht.tile([65, 512], f32, tag="ht", name=f"ht1_{b}_{qb}")
                def emit_scores_exp(kc):
                    sc = ps_sc.tile(
                        [128, 1024], f32, tag="sc", name=f"sc{b}_{qb}_{kc}"
                    )
                    ks = base + kc * 128
                    nc.tensor.matmul(
                        sc[:, 0:512],
                        kt[0:64, ks : ks + 128],
                        qt[0:64, qs : qs + 512],
                        start=True,
                        stop=True,
                    )
                    nc.tensor.matmul(
                        sc[:, 512:1024],
                        kt[64:128, ks : ks + 128],
                        qt[64:128, qs : qs + 512],
                        start=True,
                        stop=True,
                    )
                    pb = probs_pool.tile(
                        [128, 1024], bf16, tag="pb", name=f"pb{b}_{qb}_{kc}"
                    )
                    if b == B - 1:
                        # tail batch has no proj filler; halve the exp chain
                        # latency by splitting each tile over both engines
                        nc.scalar.activation(
                            pb[:, 0:512], sc[:, 0:512], AF.Exp, scale=0.125
                        )
                        nc.vector.tensor_scalar(
                            pb[:, 512:1024].bitcast(i16),
                            sc[:, 512:1024],
                            SCH_A,
                            SCH_B,
                            ALU.mult,
                            ALU.add,
                        )
                    elif kc in DVE_KCS:
                        nc.vector.tensor_scalar(
                            pb.bitcast(i16), sc, SCH_A, SCH_B, ALU.mult, ALU.add
                        )
                    else:
                        nc.scalar.activation(pb, sc, AF.Exp, scale=0.125)
                    return pb

                def emit_pv(kc, pb):
                    c0 = kc * VCOLS
                    nc.tensor.matmul(
                        ht0,
                        v2[:, c0 : c0 + WH + 1],
                        pb[:, 0:512],
                        start=(kc == 0),
                        stop=(kc == KCH - 1),
                        skip_group_check=True,
                    )
                    nc.tensor.matmul(
                        ht1,
                        v2[:, c0 + WH + 1 : c0 + VCOLS],
                        pb[:, 512:1024],
                        start=(kc == 0),
                        stop=(kc == KCH - 1),
                        skip_group_check=True,
                    )

                # Software pipeline: scores/exp run one k-chunk ahead of pv so
                # the PE always has an independent matmul pair while exp runs.
                prev_pb = None
                for kc in range(KCH):
                    if kc % 2 == 0:
                        for fn in extra_by_kg.get(kc // 2, ()):
                            fn()
                    pb = emit_scores_exp(kc)
                    if prev_pb is not None:
                        emit_pv(kc - 1, prev_pb)
                    prev_pb = pb
                emit_pv(KCH - 1, prev_pb)
                hts_t = []
                for hh, ht in ((0, ht0), (1, ht1)):
                    hts = hts_pool.tile(
                        [65, 512], f32, tag="hts", name=f"hts{b}_{qb}_{hh}"
                    )
                    if hh == 0:
                        nc.scalar.copy(hts, ht)
                    else:
                        nc.vector.tensor_copy(hts, ht)
                    hts_t.append(hts)

                # Output epilogue, split into pieces so the two heads can be
                # interleaved (one head's DVE work hides under the other's PE
                # transpose). One [128,288] tr tile per qb = ONE PSUM bank
                # holding four independent 72-col slices (2 per head).
                state = {}

                def epi_piece(hh, t, final=False):
                    if "ho" not in state:
                        state["ho"] = [
                            ho_pool.tile(
                                [128, 256], f32, tag="ho", name=f"ho{b}_{qb}_{h}"
                            )
                            for h in range(2)
                        ]
                    if final:
                        # very last epilogue: attention is over, borrow idle
                        # sc-pool banks for 4 independent slices per head so
                        # the transpose->reciprocal->mul chains fully overlap
                        key = f"trf{hh}"
                        if key not in state:
                            state[key] = ps_sc.tile(
                                [128, 1024], f32, tag="sc", name=f"trf{b}_{qb}_{hh}"
                            )
                        tr2 = state[key][:, t * 72 : t * 72 + 72]
                    else:
                        if "tr" not in state:
                            state["tr"] = ps_tr.tile(
                                [128, 288], f32, tag="tr", name=f"tr{b}_{qb}"
                            )
                        off = hh * 144 + (t % 2) * 72
                        tr2 = state["tr"][:, off : off + 72]
                    ho = state["ho"][hh]
                    hts = hts_t[hh]
                    nc.tensor.transpose(
                        tr2[:, 0:65],
                        hts[:, t * 128 : (t + 1) * 128],
                        ident[0:65, 0:65],
                    )
                    rc = rc_pool.tile(
                        [128, 1], f32, tag="rc", name=f"rc{b}_{qb}_{hh}_{t}"
                    )
                    nc.vector.reciprocal(rc, tr2[:, 64:65])
                    nc.vector.tensor_scalar_mul(
                        ho[:, t * 64 : (t + 1) * 64], tr2[:, 0:64], rc
                    )
                    if t == 3:
                        hp = hh * WH
                        dst = out_d[qs : qs + 512, hp : hp + 64].rearrange(
                            "(t p) w -> p t w", p=128
                        )
                        nc.gpsimd.dma_start(
                            out=dst, in_=ho.rearrange("p (t w) -> p t w", t=4)
                        )

                return [
                    (lambda hh=hh, t=t, **kw: epi_piece(hh, t, **kw))
                    for t in range(4)
                    for hh in range(2)
                ]

            # ---- emission: proj/v2 of batch b+1 interleaved with attention(b) ----
            for s in range(4):
                emit_proj_sblock(s, prologue=True)
                for c in range(4 * s, 4 * s + 4):
                    emit_v2_chunk(0, c)
            pending_epis = []
            for b in range(B):
                for qb in range(QB):
                    # defer the previous qb's output epilogues into this qb's
                    # kc loop: the PE transposes then overlap attention instead
                    # of stalling on the cross-engine hts-copy latency
                    extra = [(i, e) for i, e in enumerate(pending_epis)]
                    if b + 1 < B:
                        # interleave next-batch proj pieces into the kc loop so
                        # inter-group acc-bank waits hide under attention MMs
                        s_ = 4 * (b + 1) + qb
                        xts = emit_proj_xts(s_)
                        for pi in range(3):
                            extra.append(
                                (1 + 2 * pi,
                                 lambda s_=s_, pi=pi, xts=xts: emit_proj_piece(s_, pi, xts))
                            )
                        for ci in range(4):
                            c = 4 * qb + ci
                            extra.append(
                                (7, lambda bb=b + 1, c=c: emit_v2_chunk(bb, c))
                            )
                    pending_epis = emit_attention_qb(b, qb, extra)
            for e in pending_epis:
                e(final=True)

    nc.compile()
    _CACHE["nc"] = nc
    return nc


def make_in_maps(x, mask, Wq, bq, Wk, bk, Wv, bv):
    import ml_dtypes

    bf = ml_dtypes.bfloat16
    x = np.asarray(x, dtype=np.float32)
    xT = np.ascontiguousarray(x.reshape(BS, D).T.astype(bf))
    maskT = np.ascontiguousarray(
        np.asarray(mask, dtype=np.float32)
        .reshape(B, KCH, 128)
        .transpose(2, 0, 1)
        .reshape(128, B * KCH)
    )
    in_maps = []
    for c in range(NC):
        cols = slice(c * CW, (c + 1) * CW)
        in_maps.append(
            {
                "xT": xT,
                "wqT": np.ascontiguousarray(np.asarray(Wq, np.float32)[cols, :].T.astype(bf)),
                "wkT": np.ascontiguousarray(np.asarray(Wk, np.float32)[cols, :].T.astype(bf)),
                "wvT": np.ascontiguousarray(np.asarray(Wv, np.float32)[cols, :].T.astype(bf)),
                "bq": np.ascontiguousarray(np.asarray(bq, np.float32)[cols, None]),
                "bk": np.ascontiguousarray(np.asarray(bk, np.float32)[cols, None]),
                "bv": np.ascontiguousarray(np.asarray(bv, np.float32)[cols, None]),
                "maskT": maskT,
            }
        )
    return in_maps


def assemble(results):
    out = np.empty((BS, D), dtype=np.float32)
    for c in range(NC):
        out[:, c * CW : (c + 1) * CW] = results[c]["h_out"]
    return out.reshape(B, S, D)


def kernel(x, mask, Wq, bq, Wk, bk, Wv, bv, **run_kwargs):
    _ensure_import()
    from concourse.bass_utils import run_bass_kernel_spmd

    nc = build_bass()
    in_maps = make_in_maps(x, mask, Wq, bq, Wk, bk, Wv, bv)
    res = run_bass_kernel_spmd(nc, in_maps, core_ids=list(range(NC)), **run_kwargs)
    _CACHE["last_results"] = res
    return assemble(res.results)


# revision 67
# speedup vs baseline: 1.0955x; 1.0139x over previous
# Multi-headed self-attention (B=4, S=2048, D=1024, H=16) on 8 TRN2 NeuronCores.
#
# Sharding: tensor-parallel over heads. Core c computes heads 2c, 2c+1 (=128
# output columns) for all batches. Host pre-transposes x -> xT [D, B*S] and the
# per-core weight slices -> [D, 128] so every matmul contracts over the
# partition dimension. Host gathers the 8 [B*S, 128] outputs into (B,S,D).
#
# Per-core dataflow (bf16 PE operands, fp32 PSUM accumulation):
#   1. Projections (bf16 x, bf16 W): QT/KT [128(2 heads x 64), 8192] bf16 and
#      VT f32, accumulated over 8 d-chunks in PSUM; bias added during the
#      PSUM->SBUF move (DVE per-partition scalar add).
#   2. V2 prep: one packed PE transpose per 128-t chunk turns VT[128(2 heads),
#      128t] into [128t, 128w]; DVE applies the key mask and appends a mask
#      column per head -> v2 chunk layout [V_h0(64)|m|V_h1(64)|m] (130 cols,
#      bf16).
#   3. Attention per (batch, q-block): per k-chunk ONE [128, 1024] PSUM tile
#      holds both heads' scoresT [128 k, 512 q]; the two K=64 bf16 matmuls
#      sit on disjoint PE row halves (partitions 0:64 / 64:128) and their
#      moving streams overlap (~1.4 cols/cycle aggregate, beating the 1
#      col/cycle single-matmul floor). exp alternates engines per k-chunk:
#      ScalarE exact Exp (fused 1/8 scale) for 9/16, VectorE Schraudolph
#      bit-trick exp (one tensor_scalar: bf16 bits = int16(x*A+B), ~1.8% rel
#      err, softmax-safe since the denominator uses the same approximation)
#      for 7/16 — exactly one VectorE chunk per scores-PSUM pair so the two
#      engines always run concurrently. No row-max subtraction (scores std
#      ~0.4, exp is safe, softmax is shift-invariant). Scores/exp run one
#      k-chunk ahead of the PV matmuls (software pipeline). PV matmuls use
#      the 65-col bf16 stationary [V|mask] so the accumulation yields
#      unnormalized h^T plus the softmax denominator. PE-transpose h''^T back
#      to [q, 65], DVE reciprocal of column 64, per-partition scalar
#      multiply, DMA out. hts PSUM->SBUF copies split ScalarE/VectorE.
#   The 0/1 mask is exact this way: reference's exp(-10000) == 0.0 in fp32.
#   Emission interleaves proj/v2-prep of batch b+1 with attention of batch b
#   to keep the PE dense; each q-block's output epilogue (transpose/normalize/
#   DMA) is deferred into the NEXT q-block's kc loop so the PE transposes
#   overlap attention instead of stalling on cross-engine copy latency.
#   bf16 quantization of x/W/Q/K/V/probs plus the Schraudolph share puts the
#   end-to-end rel error at ~8e-3 (gate 2e-2).

import sys

import numpy as np

B, S, D, H = 4, 2048, 1024, 16
NC = 8
HPC = H // NC  # heads per core = 2
WH = D // H  # head width = 64
CW = HPC * WH  # per-core output width = 128
BS = B * S  # 8192
DCH = D // 128  # d chunks = 8
QB = S // 512  # q blocks per batch = 4
KCH = S // 128  # k chunks per batch = 16
VCOLS = 2 * (WH + 1)  # v2 chunk cols = 130

# Schraudolph exp in bf16: exp(x*0.125) ~= bitcast_bf16(int16(x*A + Bc))
# (bf16 = 8-bit exp, 7-bit mantissa -> the int domain is 2^7 per octave)
_LN2 = float(np.log(2.0))
SCH_A = 0.125 * (2**7) / _LN2
SCH_B = 127.0 * (2**7) - 5.5
# k-chunks whose exp runs on VectorE (Schraudolph); rest on ScalarE (exact).
DVE_KCS = frozenset({1, 3, 5, 8, 10, 13, 15})

_CACHE = {}


def _ensure_import():
    try:
        import concourse.bass  # noqa: F401
    except ImportError:
        sys.path.insert(0, "/opt/trn_rl_repo")
        import concourse.bass  # noqa: F401


def build_bass():
    if "nc" in _CACHE:
        return _CACHE["nc"]
    _ensure_import()
    import concourse.mybir as mybir
    import concourse.tile as tile
    from concourse import bacc
    from concourse.masks import make_identity

    f32 = mybir.dt.float32
    f32r = mybir.dt.float32r
    bf16 = mybir.dt.bfloat16
    i16 = mybir.dt.int16
    AF = mybir.ActivationFunctionType
    ALU = mybir.AluOpType

    nc = bacc.Bacc(
        "TRN2",
        target_bir_lowering=False,
        debug=False,
        enable_asserts=False,
        num_devices=NC,
    )
    xT_d = nc.dram_tensor("xT", (D, BS), bf16, kind="ExternalInput").ap()
    wq_d = nc.dram_tensor("wqT", (D, CW), bf16, kind="ExternalInput").ap()
    wk_d = nc.dram_tensor("wkT", (D, CW), bf16, kind="ExternalInput").ap()
    wv_d = nc.dram_tensor("wvT", (D, CW), bf16, kind="ExternalInput").ap()
    bq_d = nc.dram_tensor("bq", (CW, 1), f32, kind="ExternalInput").ap()
    bk_d = nc.dram_tensor("bk", (CW, 1), f32, kind="ExternalInput").ap()
    bv_d = nc.dram_tensor("bv", (CW, 1), f32, kind="ExternalInput").ap()
    mask_d = nc.dram_tensor("maskT", (128, B * KCH), f32, kind="ExternalInput").ap()
    out_d = nc.dram_tensor("h_out", (BS, CW), f32, kind="ExternalOutput").ap()

    with tile.TileContext(nc) as tc:
        with (
            tc.tile_pool(name="qkv", bufs=1) as qkv_pool,
            tc.tile_pool(name="xt", bufs=20) as xt_pool,
            tc.tile_pool(name="wsb", bufs=1) as w_pool,
            tc.tile_pool(name="probs", bufs=8) as probs_pool,
            tc.tile_pool(name="v2", bufs=2) as v2_pool,
            tc.tile_pool(name="hts", bufs=6) as hts_pool,
            tc.tile_pool(name="ho", bufs=4) as ho_pool,
            tc.tile_pool(name="rc", bufs=16) as rc_pool,
            tc.tile_pool(name="cst", bufs=1) as cst_pool,
            tc.tile_pool(name="ps_sc", bufs=2, space="PSUM") as ps_sc,
            tc.tile_pool(name="ps_ht", bufs=2, space="PSUM") as ps_ht,
            tc.tile_pool(name="ps_acc", bufs=1, space="PSUM") as ps_acc,
            tc.tile_pool(name="ps_tr", bufs=1, space="PSUM") as ps_tr,
        ):
            ident = cst_pool.tile([128, 128], f32, tag="ident")
            make_identity(nc, ident)



            wsbs = []
            for name, dram in (("wq", wq_d), ("wk", wk_d), ("wv", wv_d)):
                w_sb = w_pool.tile([128, DCH * CW], bf16, tag=name)
                nc.sync.dma_start(
                    out=w_sb.rearrange("p (c w) -> p c w", c=DCH),
                    in_=dram.rearrange("(c p) w -> p c w", p=128),
                )
                wsbs.append(w_sb)
            bsbs = []
            for name, dram in (("bq", bq_d), ("bk", bk_d), ("bv", bv_d)):
                b_sb = cst_pool.tile([128, 1], f32, tag=name)
                nc.sync.dma_start(out=b_sb, in_=dram)
                bsbs.append(b_sb)
            mask_sb = cst_pool.tile([128, B * KCH], f32, tag="mask")
            nc.sync.dma_start(out=mask_sb, in_=mask_d)

            qt = qkv_pool.tile([128, BS], bf16, tag="qt")
            kt = qkv_pool.tile([128, BS], bf16, tag="kt")
            vt = qkv_pool.tile([128, BS], f32, tag="vt")
            qkv_sb = [qt, kt, vt]

            v2_tiles = {}

            def emit_proj_xts(s_):
                xts = []
                for d in range(DCH):
                    xt_t = xt_pool.tile([128, 512], bf16, tag="xt", name=f"xt{s_}_{d}")
                    nc.sync.dma_start(
                        out=xt_t,
                        in_=xT_d[d * 128 : (d + 1) * 128, s_ * 512 : (s_ + 1) * 512],
                    )
                    xts.append(xt_t)
                return xts

            def emit_proj_piece(s_, pi, xts, prologue=False):
                if prologue:
                    # attention hasn't started: borrow the idle sc-pool banks
                    # so prologue proj groups double-buffer instead of
                    # stalling on the single acc bank
                    acc = ps_sc.tile(
                        [128, 1024], f32, tag="sc", name=f"pjp{s_}_{pi}"
                    )[:, 0:512]
                else:
                    acc = ps_acc.tile([128, 512], f32, tag="acc", name=f"pj{s_}_{pi}")
                w_sb = wsbs[pi]
                for d in range(DCH):
                    nc.tensor.matmul(
                        acc,
                        w_sb[:, d * CW : (d + 1) * CW],
                        xts[d],
                        start=(d == 0),
                        stop=(d == DCH - 1),
                    )
                dst = qkv_sb[pi][:, s_ * 512 : (s_ + 1) * 512]
                nc.vector.tensor_scalar_add(dst, acc, bsbs[pi])

            def emit_proj_sblock(s_, prologue=False):
                xts = emit_proj_xts(s_)
                for pi in range(3):
                    emit_proj_piece(s_, pi, xts, prologue=prologue)

            def emit_v2_chunk(b, i):
                # One packed transpose: VT[128(2 heads x 64w), 128t] -> [128t, 128w].
                if (b, 0) not in v2_tiles:
                    v2 = v2_pool.tile([128, KCH * VCOLS], bf16, tag="v2", name=f"v2_{b}")
                    v2_tiles[(b, 0)] = v2
                v2 = v2_tiles[(b, 0)]
                vtr_full = ps_acc.tile([128, 512], f32, tag="acc", name=f"vtr{b}_{i}")
                vtr = vtr_full[:, 0:128]
                nc.tensor.transpose(
                    vtr, vt[:, b * S + i * 128 : b * S + (i + 1) * 128], ident
                )
                mcol = mask_sb[:, b * KCH + i : b * KCH + i + 1]
                ch = v2[:, i * VCOLS : (i + 1) * VCOLS]
                ch2 = ch.rearrange("p (g w) -> p g w", g=2)
                vtr2 = vtr.rearrange("p (g w) -> p g w", g=2)
                nc.vector.tensor_scalar_mul(ch2[:, :, 0:WH], vtr2, mcol)
                nc.vector.tensor_copy(ch[:, WH : WH + 1], mcol)
                nc.vector.tensor_copy(ch[:, VCOLS - 1 : VCOLS], mcol)

            def emit_attention_qb(b, qb, extra=()):
                # `extra`: list of (kg_slot, fn) emitted at the top of that kg
                # iteration — used to interleave next-batch proj/v2 PE work so
                # PSUM-drain latencies hide under attention matmuls.
                extra_by_kg = {}
                for slot, fn in extra:
                    extra_by_kg.setdefault(slot, []).append(fn)
                v2 = v2_tiles[(b, 0)]
                base = b * S
                qs = base + qb * 512
                ht0 = ps_ht.tile([65, 512], f32, tag="ht", name=f"ht0_{b}_{qb}")
                ht1 = ps_ht.tile([65, 512], f32, tag="ht", name=f"ht1_{b}_{qb}")
                def emit_scores_exp(kc):
                    sc = ps_sc.tile(
                        [128, 1024], f32, tag="sc", name=f"sc{b}_{qb}_{kc}"
                    )
                    ks = base + kc * 128
                    nc.tensor.matmul(
                        sc[:, 0:512],
                        kt[0:64, ks : ks + 128],
                        qt[0:64, qs : qs + 512],
                        start=True,
                        stop=True,
                    )
                    nc.tensor.matmul(
                        sc[:, 512:1024],
                        kt[64:128, ks : ks + 128],
                        qt[64:128, qs : qs + 512],
                        start=True,
                        stop=True,
                    )
                    pb = probs_pool.tile(
                        [128, 1024], bf16, tag="pb", name=f"pb{b}_{qb}_{kc}"
                    )
                    if b == B - 1:
                        # tail batch has no proj filler; halve the exp chain
                        # latency by splitting each tile over both engines
                        nc.scalar.activation(
                            pb[:, 0:512], sc[:, 0:512], AF.Exp, scale=0.125
                        )
                        nc.vector.tensor_scalar(
                            pb[:, 512:1024].bitcast(i16),
                            sc[:, 512:1024],
                            SCH_A,
                            SCH_B,
                            ALU.mult,
                            ALU.add,
                        )
                    elif kc in DVE_KCS:
                        nc.vector.tensor_scalar(
                            pb.bitcast(i16), sc, SCH_A, SCH_B, ALU.mult, ALU.add
                        )
                    else:
                        nc.scalar.activation(pb, sc, AF.Exp, scale=0.125)
                    return pb

                def emit_pv(kc, pb):
                    c0 = kc * VCOLS
                    nc.tensor.matmul(
                        ht0,
                        v2[:, c0 : c0 + WH + 1],
                        pb[:, 0:512],
                        start=(kc == 0),
                        stop=(kc == KCH - 1),
                        skip_group_check=True,
                    )
                    nc.tensor.matmul(
                        ht1,
                        v2[:, c0 + WH + 1 : c0 + VCOLS],
                        pb[:, 512:1024],
                        start=(kc == 0),
                        stop=(kc == KCH - 1),
                        skip_group_check=True,
                    )

                # Software pipeline: scores/exp run one k-chunk ahead of pv so
                # the PE always has an independent matmul pair while exp runs.
                prev_pb = None
                for kc in range(KCH):
                    if kc % 2 == 0:
                        for fn in extra_by_kg.get(kc // 2, ()):
                            fn()
                    pb = emit_scores_exp(kc)
                    if prev_pb is not None:
                        emit_pv(kc - 1, prev_pb)
                    prev_pb = pb
                emit_pv(KCH - 1, prev_pb)
                hts_t = []
                for hh, ht in ((0, ht0), (1, ht1)):
                    hts = hts_pool.tile(
                        [65, 512], f32, tag="hts", name=f"hts{b}_{qb}_{hh}"
                    )
                    if hh == 0:
                        nc.scalar.copy(hts, ht)
                    else:
                        nc.vector.tensor_copy(hts, ht)
                    hts_t.append(hts)

                # Output epilogue, split into pieces so the two heads can be
                # interleaved (one head's DVE work hides under the other's PE
                # transpose). One [128,288] tr tile per qb = ONE PSUM bank
                # holding four independent 72-col slices (2 per head).
                state = {}

                def epi_piece(hh, t, final=False):
                    if "ho" not in state:
                        state["ho"] = [
                            ho_pool.tile(
                                [128, 256], f32, tag="ho", name=f"ho{b}_{qb}_{h}"
                            )
                            for h in range(2)
                        ]
                    if final:
                        # very last epilogue: attention is over, borrow idle
                        # sc-pool banks for 4 independent slices per head so
                        # the transpose->reciprocal->mul chains fully overlap
                        key = f"trf{hh}"
                        if key not in state:
                            state[key] = ps_sc.tile(
                                [128, 1024], f32, tag="sc", name=f"trf{b}_{qb}_{hh}"
                            )
                        tr2 = state[key][:, t * 72 : t * 72 + 72]
                    else:
                        if "tr" not in state:
                            state["tr"] = ps_tr.tile(
                                [128, 288], f32, tag="tr", name=f"tr{b}_{qb}"
                            )
                        off = hh * 144 + (t % 2) * 72
                        tr2 = state["tr"][:, off : off + 72]
                    ho = state["ho"][hh]
                    hts = hts_t[hh]
                    nc.tensor.transpose(
                        tr2[:, 0:65],
                        hts[:, t * 128 : (t + 1) * 128],
                        ident[0:65, 0:65],
                    )
                    rc = rc_pool.tile(
                        [128, 1], f32, tag="rc", name=f"rc{b}_{qb}_{hh}_{t}"
                    )
                    nc.vector.reciprocal(rc, tr2[:, 64:65])
                    nc.vector.tensor_scalar_mul(
                        ho[:, t * 64 : (t + 1) * 64], tr2[:, 0:64], rc
                    )
                    if t == 3:
                        hp = hh * WH
                        dst = out_d[qs : qs + 512, hp : hp + 64].rearrange(
                            "(t p) w -> p t w", p=128
                        )
                        nc.gpsimd.dma_start(
                            out=dst, in_=ho.rearrange("p (t w) -> p t w", t=4)
                        )

                return [
                    (lambda hh=hh, t=t, **kw: epi_piece(hh, t, **kw))
                    for t in range(4)
                    for hh in range(2)
                ]

            # ---- emission: proj/v2 of batch b+1 interleaved with attention(b) ----
            for s in range(4):
                emit_proj_sblock(s, prologue=True)
                for c in range(4 * s, 4 * s + 4):
                    emit_v2_chunk(0, c)
            pending_epis = []
            for b in range(B):
                for qb in range(QB):
                    # defer the previous qb's output epilogues into this qb's
                    # kc loop: the PE transposes then overlap attention instead
                    # of stalling on the cross-engine hts-copy latency
                    extra = [(i, e) for i, e in enumerate(pending_epis)]
                    if b + 1 < B:
                        # interleave next-batch proj pieces into the kc loop so
                        # inter-group acc-bank waits hide under attention MMs
                        s_ = 4 * (b + 1) + qb
                        xts = emit_proj_xts(s_)
                        for pi in range(3):
                            extra.append(
                                (1 + 2 * pi,
                                 lambda s_=s_, pi=pi, xts=xts: emit_proj_piece(s_, pi, xts))
                            )
                        for ci in range(4):
                            c = 4 * qb + ci
                            extra.append(
                                (6 + ci // 2, lambda bb=b + 1, c=c: emit_v2_chunk(bb, c))
                            )
                    pending_epis = emit_attention_qb(b, qb, extra)
            for e in pending_epis:
                e(final=True)

    nc.compile()
    _CACHE["nc"] = nc
    return nc


def make_in_maps(x, mask, Wq, bq, Wk, bk, Wv, bv):
    import ml_dtypes

    bf = ml_dtypes.bfloat16
    x = np.asarray(x, dtype=np.float32)
    xT = np.ascontiguousarray(x.reshape(BS, D).T.astype(bf))
    maskT = np.ascontiguousarray(
        np.asarray(mask, dtype=np.float32)
        .reshape(B, KCH, 128)
        .transpose(2, 0, 1)
        .reshape(128, B * KCH)
    )
    in_maps = []
    for c in range(NC):
        cols = slice(c * CW, (c + 1) * CW)
        in_maps.append(
            {
                "xT": xT,
                "wqT": np.ascontiguousarray(np.asarray(Wq, np.float32)[cols, :].T.astype(bf)),
                "wkT": np.ascontiguousarray(np.asarray(Wk, np.float32)[cols, :].T.astype(bf)),
                "wvT": np.ascontiguousarray(np.asarray(Wv, np.float32)[cols, :].T.astype(bf)),
                "bq": np.ascontiguousarray(np.asarray(bq, np.float32)[cols, None]),
                "bk": np.ascontiguousarray(np.asarray(bk, np.float32)[cols, None]),
                "bv": np.ascontiguousarray(np.asarray(bv, np.float32)[cols, None]),
                "maskT": maskT,
            }
        )
    return in_maps


def assemble(results):
    out = np.empty((BS, D), dtype=np.float32)
    for c in range(NC):
        out[:, c * CW : (c + 1) * CW] = results[c]["h_out"]
    return out.reshape(B, S, D)


def kernel(x, mask, Wq, bq, Wk, bk, Wv, bv, **run_kwargs):
    _ensure_import()
    from concourse.bass_utils import run_bass_kernel_spmd

    nc = build_bass()
    in_maps = make_in_maps(x, mask, Wq, bq, Wk, bk, Wv, bv)
    res = run_bass_kernel_spmd(nc, in_maps, core_ids=list(range(NC)), **run_kwargs)
    _CACHE["last_results"] = res
    return assemble(res.results)


# revision 75
# speedup vs baseline: 1.0967x; 1.0011x over previous
# Multi-headed self-attention (B=4, S=2048, D=1024, H=16) on 8 TRN2 NeuronCores.
#
# Sharding: tensor-parallel over heads. Core c computes heads 2c, 2c+1 (=128
# output columns) for all batches. Host pre-transposes x -> xT [D, B*S] and the
# per-core weight slices -> [D, 128] so every matmul contracts over the
# partition dimension. Host gathers the 8 [B*S, 128] outputs into (B,S,D).
#
# Per-core dataflow (bf16 PE operands, fp32 PSUM accumulation):
#   1. Projections (bf16 x, bf16 W): QT/KT [128(2 heads x 64), 8192] bf16 and
#      VT f32, accumulated over 8 d-chunks in PSUM; bias added during the
#      PSUM->SBUF move (DVE per-partition scalar add).
#   2. V2 prep: one packed PE transpose per 128-t chunk turns VT[128(2 heads),
#      128t] into [128t, 128w]; DVE applies the key mask and appends a mask
#      column per head -> v2 chunk layout [V_h0(64)|m|V_h1(64)|m] (130 cols,
#      bf16).
#   3. Attention per (batch, q-block): per k-chunk ONE [128, 1024] PSUM tile
#      holds both heads' scoresT [128 k, 512 q]; the two K=64 bf16 matmuls
#      sit on disjoint PE row halves (partitions 0:64 / 64:128) and their
#      moving streams overlap (~1.4 cols/cycle aggregate, beating the 1
#      col/cycle single-matmul floor). exp alternates engines per k-chunk:
#      ScalarE exact Exp (fused 1/8 scale) for 9/16, VectorE Schraudolph
#      bit-trick exp (one tensor_scalar: bf16 bits = int16(x*A+B), ~1.8% rel
#      err, softmax-safe since the denominator uses the same approximation)
#      for 7/16 — exactly one VectorE chunk per scores-PSUM pair so the two
#      engines always run concurrently. No row-max subtraction (scores std
#      ~0.4, exp is safe, softmax is shift-invariant). Scores/exp run one
#      k-chunk ahead of the PV matmuls (software pipeline). PV matmuls use
#      the 65-col bf16 stationary [V|mask] so the accumulation yields
#      unnormalized h^T plus the softmax denominator. PE-transpose h''^T back
#      to [q, 65], DVE reciprocal of column 64, per-partition scalar
#      multiply, DMA out. hts PSUM->SBUF copies split ScalarE/VectorE.
#   The 0/1 mask is exact this way: reference's exp(-10000) == 0.0 in fp32.
#   Emission interleaves proj/v2-prep of batch b+1 with attention of batch b
#   to keep the PE dense; each q-block's output epilogue (transpose/normalize/
#   DMA) is deferred into the NEXT q-block's kc loop so the PE transposes
#   overlap attention instead of stalling on cross-engine copy latency.
#   bf16 quantization of x/W/Q/K/V/probs plus the Schraudolph share puts the
#   end-to-end rel error at ~8e-3 (gate 2e-2).

import sys

import numpy as np

B, S, D, H = 4, 2048, 1024, 16
NC = 8
HPC = H // NC  # heads per core = 2
WH = D // H  # head width = 64
CW = HPC * WH  # per-core output width = 128
BS = B * S  # 8192
DCH = D // 128  # d chunks = 8
QB = S // 512  # q blocks per batch = 4
KCH = S // 128  # k chunks per batch = 16
VCOLS = 2 * (WH + 1)  # v2 chunk cols = 130

# Schraudolph exp in bf16: exp(x*0.125) ~= bitcast_bf16(int16(x*A + Bc))
# (bf16 = 8-bit exp, 7-bit mantissa -> the int domain is 2^7 per octave)
_LN2 = float(np.log(2.0))
SCH_A = 0.125 * (2**7) / _LN2
SCH_B = 127.0 * (2**7) - 5.5
# k-chunks whose exp runs on VectorE (Schraudolph); rest on ScalarE (exact).
DVE_KCS = frozenset({1, 3, 5, 8, 10, 13, 15})

_CACHE = {}


def _ensure_import():
    try:
        import concourse.bass  # noqa: F401
    except ImportError:
        sys.path.insert(0, "/opt/trn_rl_repo")
        import concourse.bass  # noqa: F401


def build_bass():
    if "nc" in _CACHE:
        return _CACHE["nc"]
    _ensure_import()
    import concourse.mybir as mybir
    import concourse.tile as tile
    from concourse import bacc
    from concourse.masks import make_identity

    f32 = mybir.dt.float32
    f32r = mybir.dt.float32r
    bf16 = mybir.dt.bfloat16
    i16 = mybir.dt.int16
    AF = mybir.ActivationFunctionType
    ALU = mybir.AluOpType

    nc = bacc.Bacc(
        "TRN2",
        target_bir_lowering=False,
        debug=False,
        enable_asserts=False,
        num_devices=NC,
    )
    xT_d = nc.dram_tensor("xT", (D, BS), bf16, kind="ExternalInput").ap()
    # weights arrive pre-arranged in SBUF layout [128, DCH*CW] so the load is
    # 128 contiguous 2KB descriptors instead of 1024 strided 256B ones
    wq_d = nc.dram_tensor("wqT", (128, DCH * CW), bf16, kind="ExternalInput").ap()
    wk_d = nc.dram_tensor("wkT", (128, DCH * CW), bf16, kind="ExternalInput").ap()
    wv_d = nc.dram_tensor("wvT", (128, DCH * CW), bf16, kind="ExternalInput").ap()
    bq_d = nc.dram_tensor("bq", (CW, 1), f32, kind="ExternalInput").ap()
    bk_d = nc.dram_tensor("bk", (CW, 1), f32, kind="ExternalInput").ap()
    bv_d = nc.dram_tensor("bv", (CW, 1), f32, kind="ExternalInput").ap()
    mask_d = nc.dram_tensor("maskT", (128, B * KCH), f32, kind="ExternalInput").ap()
    out_d = nc.dram_tensor("h_out", (BS, CW), f32, kind="ExternalOutput").ap()

    with tile.TileContext(nc) as tc:
        with (
            tc.tile_pool(name="qkv", bufs=1) as qkv_pool,
            tc.tile_pool(name="xt", bufs=20) as xt_pool,
            tc.tile_pool(name="wsb", bufs=1) as w_pool,
            tc.tile_pool(name="probs", bufs=8) as probs_pool,
            tc.tile_pool(name="v2", bufs=2) as v2_pool,
            tc.tile_pool(name="hts", bufs=6) as hts_pool,
            tc.tile_pool(name="ho", bufs=4) as ho_pool,
            tc.tile_pool(name="rc", bufs=16) as rc_pool,
            tc.tile_pool(name="cst", bufs=1) as cst_pool,
            tc.tile_pool(name="ps_sc", bufs=2, space="PSUM") as ps_sc,
            tc.tile_pool(name="ps_ht", bufs=2, space="PSUM") as ps_ht,
            tc.tile_pool(name="ps_acc", bufs=1, space="PSUM") as ps_acc,
            tc.tile_pool(name="ps_tr", bufs=1, space="PSUM") as ps_tr,
        ):
            ident = cst_pool.tile([128, 128], f32, tag="ident")
            make_identity(nc, ident)



            wsbs = []
            for name, dram in (("wq", wq_d), ("wk", wk_d), ("wv", wv_d)):
                w_sb = w_pool.tile([128, DCH * CW], bf16, tag=name)
                nc.sync.dma_start(out=w_sb, in_=dram)
                wsbs.append(w_sb)
            bsbs = []
            for name, dram in (("bq", bq_d), ("bk", bk_d), ("bv", bv_d)):
                b_sb = cst_pool.tile([128, 1], f32, tag=name)
                nc.sync.dma_start(out=b_sb, in_=dram)
                bsbs.append(b_sb)
            mask_sb = cst_pool.tile([128, B * KCH], f32, tag="mask")
            nc.sync.dma_start(out=mask_sb, in_=mask_d)

            qt = qkv_pool.tile([128, BS], bf16, tag="qt")
            kt = qkv_pool.tile([128, BS], bf16, tag="kt")
            vt = qkv_pool.tile([128, BS], f32, tag="vt")
            qkv_sb = [qt, kt, vt]

            v2_tiles = {}

            def emit_proj_xts(s_):
                xts = []
                for d in range(DCH):
                    xt_t = xt_pool.tile([128, 512], bf16, tag="xt", name=f"xt{s_}_{d}")
                    nc.sync.dma_start(
                        out=xt_t,
                        in_=xT_d[d * 128 : (d + 1) * 128, s_ * 512 : (s_ + 1) * 512],
                    )
                    xts.append(xt_t)
                return xts

            def emit_proj_piece(s_, pi, xts, prologue=False):
                if prologue:
                    # attention hasn't started: borrow the idle sc-pool banks
                    # so prologue proj groups double-buffer instead of
                    # stalling on the single acc bank
                    acc = ps_sc.tile(
                        [128, 1024], f32, tag="sc", name=f"pjp{s_}_{pi}"
                    )[:, 0:512]
                else:
                    acc = ps_acc.tile([128, 512], f32, tag="acc", name=f"pj{s_}_{pi}")
                w_sb = wsbs[pi]
                for d in range(DCH):
                    nc.tensor.matmul(
                        acc,
                        w_sb[:, d * CW : (d + 1) * CW],
                        xts[d],
                        start=(d == 0),
                        stop=(d == DCH - 1),
                    )
                dst = qkv_sb[pi][:, s_ * 512 : (s_ + 1) * 512]
                nc.vector.tensor_scalar_add(dst, acc, bsbs[pi])

            def emit_proj_sblock(s_, prologue=False):
                xts = emit_proj_xts(s_)
                for pi in range(3):
                    emit_proj_piece(s_, pi, xts, prologue=prologue)

            def emit_v2_chunk(b, i):
                # One packed transpose: VT[128(2 heads x 64w), 128t] -> [128t, 128w].
                if (b, 0) not in v2_tiles:
                    v2 = v2_pool.tile([128, KCH * VCOLS], bf16, tag="v2", name=f"v2_{b}")
                    v2_tiles[(b, 0)] = v2
                v2 = v2_tiles[(b, 0)]
                vtr_full = ps_acc.tile([128, 512], f32, tag="acc", name=f"vtr{b}_{i}")
                vtr = vtr_full[:, 0:128]
                nc.tensor.transpose(
                    vtr, vt[:, b * S + i * 128 : b * S + (i + 1) * 128], ident
                )
                mcol = mask_sb[:, b * KCH + i : b * KCH + i + 1]
                ch = v2[:, i * VCOLS : (i + 1) * VCOLS]
                ch2 = ch.rearrange("p (g w) -> p g w", g=2)
                vtr2 = vtr.rearrange("p (g w) -> p g w", g=2)
                nc.vector.tensor_scalar_mul(ch2[:, :, 0:WH], vtr2, mcol)
                nc.vector.tensor_copy(ch[:, WH : WH + 1], mcol)
                nc.vector.tensor_copy(ch[:, VCOLS - 1 : VCOLS], mcol)

            def emit_attention_qb(b, qb, extra=()):
                # `extra`: list of (kg_slot, fn) emitted at the top of that kg
                # iteration — used to interleave next-batch proj/v2 PE work so
                # PSUM-drain latencies hide under attention matmuls.
                extra_by_kg = {}
                for slot, fn in extra:
                    extra_by_kg.setdefault(slot, []).append(fn)
                v2 = v2_tiles[(b, 0)]
                base = b * S
                qs = base + qb * 512
                ht0 = ps_ht.tile([65, 512], f32, tag="ht", name=f"ht0_{b}_{qb}")
                ht1 = ps_ht.tile([65, 512], f32, tag="ht", name=f"ht1_{b}_{qb}")
                def emit_scores_exp(kc):
                    sc = ps_sc.tile(
                        [128, 1024], f32, tag="sc", name=f"sc{b}_{qb}_{kc}"
                    )
                    ks = base + kc * 128
                    nc.tensor.matmul(
                        sc[:, 0:512],
                        kt[0:64, ks : ks + 128],
                        qt[0:64, qs : qs + 512],
                        start=True,
                        stop=True,
                    )
                    nc.tensor.matmul(
                        sc[:, 512:1024],
                        kt[64:128, ks : ks + 128],
                        qt[64:128, qs : qs + 512],
                        start=True,
                        stop=True,
                    )
                    pb = probs_pool.tile(
                        [128, 1024], bf16, tag="pb", name=f"pb{b}_{qb}_{kc}"
                    )
                    if b == B - 1:
                        # tail batch has no proj filler; halve the exp chain
                        # latency by splitting each tile over both engines
                        nc.scalar.activation(
                            pb[:, 0:512], sc[:, 0:512], AF.Exp, scale=0.125
                        )
                        nc.vector.tensor_scalar(
                            pb[:, 512:1024].bitcast(i16),
                            sc[:, 512:1024],
                            SCH_A,
                            SCH_B,
                            ALU.mult,
                            ALU.add,
                        )
                    elif kc in DVE_KCS:
                        nc.vector.tensor_scalar(
                            pb.bitcast(i16), sc, SCH_A, SCH_B, ALU.mult, ALU.add
                        )
                    else:
                        nc.scalar.activation(pb, sc, AF.Exp, scale=0.125)
                    return pb

                def emit_pv(kc, pb):
                    c0 = kc * VCOLS
                    nc.tensor.matmul(
                        ht0,
                        v2[:, c0 : c0 + WH + 1],
                        pb[:, 0:512],
                        start=(kc == 0),
                        stop=(kc == KCH - 1),
                        skip_group_check=True,
                    )
                    nc.tensor.matmul(
                        ht1,
                        v2[:, c0 + WH + 1 : c0 + VCOLS],
                        pb[:, 512:1024],
                        start=(kc == 0),
                        stop=(kc == KCH - 1),
                        skip_group_check=True,
                    )

                # Software pipeline: scores/exp run one k-chunk ahead of pv so
                # the PE always has an independent matmul pair while exp runs.
                prev_pb = None
                for kc in range(KCH):
                    if kc % 2 == 0:
                        for fn in extra_by_kg.get(kc // 2, ()):
                            fn()
                    pb = emit_scores_exp(kc)
                    if prev_pb is not None:
                        emit_pv(kc - 1, prev_pb)
                    prev_pb = pb
                emit_pv(KCH - 1, prev_pb)
                hts_t = []
                for hh, ht in ((0, ht0), (1, ht1)):
                    hts = hts_pool.tile(
                        [65, 512], f32, tag="hts", name=f"hts{b}_{qb}_{hh}"
                    )
                    if hh == 0:
                        nc.scalar.copy(hts, ht)
                    else:
                        nc.vector.tensor_copy(hts, ht)
                    hts_t.append(hts)

                # Output epilogue, split into pieces so the two heads can be
                # interleaved (one head's DVE work hides under the other's PE
                # transpose). One [128,288] tr tile per qb = ONE PSUM bank
                # holding four independent 72-col slices (2 per head).
                state = {}

                def epi_piece(hh, t, final=False):
                    if "ho" not in state:
                        state["ho"] = [
                            ho_pool.tile(
                                [128, 256], f32, tag="ho", name=f"ho{b}_{qb}_{h}"
                            )
                            for h in range(2)
                        ]
                    if final:
                        # very last epilogue: attention is over, borrow idle
                        # sc-pool banks for 4 independent slices per head so
                        # the transpose->reciprocal->mul chains fully overlap
                        key = f"trf{hh}"
                        if key not in state:
                            state[key] = ps_sc.tile(
                                [128, 1024], f32, tag="sc", name=f"trf{b}_{qb}_{hh}"
                            )
                        tr2 = state[key][:, t * 72 : t * 72 + 72]
                    else:
                        if "tr" not in state:
                            state["tr"] = ps_tr.tile(
                                [128, 288], f32, tag="tr", name=f"tr{b}_{qb}"
                            )
                        off = hh * 144 + (t % 2) * 72
                        tr2 = state["tr"][:, off : off + 72]
                    ho = state["ho"][hh]
                    hts = hts_t[hh]
                    nc.tensor.transpose(
                        tr2[:, 0:65],
                        hts[:, t * 128 : (t + 1) * 128],
                        ident[0:65, 0:65],
                    )
                    rc = rc_pool.tile(
                        [128, 1], f32, tag="rc", name=f"rc{b}_{qb}_{hh}_{t}"
                    )
                    nc.vector.reciprocal(rc, tr2[:, 64:65])
                    nc.vector.tensor_scalar_mul(
                        ho[:, t * 64 : (t + 1) * 64], tr2[:, 0:64], rc
                    )
                    if t == 3:
                        hp = hh * WH
                        dst = out_d[qs : qs + 512, hp : hp + 64].rearrange(
                            "(t p) w -> p t w", p=128
                        )
                        nc.gpsimd.dma_start(
                            out=dst, in_=ho.rearrange("p (t w) -> p t w", t=4)
                        )

                return [
                    (lambda hh=hh, t=t, **kw: epi_piece(hh, t, **kw))
                    for t in range(4)
                    for hh in range(2)
                ]

            # ---- emission: proj/v2 of batch b+1 interleaved with attention(b) ----
            for s in range(4):
                emit_proj_sblock(s, prologue=True)
                for c in range(4 * s, 4 * s + 4):
                    emit_v2_chunk(0, c)
            pending_epis = []
            for b in range(B):
                for qb in range(QB):
                    # defer the previous qb's output epilogues into this qb's
                    # kc loop: the PE transposes then overlap attention instead
                    # of stalling on the cross-engine hts-copy latency
                    extra = [(i, e) for i, e in enumerate(pending_epis)]
                    if b + 1 < B:
                        # interleave next-batch proj pieces into the kc loop so
                        # inter-group acc-bank waits hide under attention MMs
                        s_ = 4 * (b + 1) + qb
                        xts = emit_proj_xts(s_)
                        for pi in range(3):
                            extra.append(
                                (1 + 2 * pi,
                                 lambda s_=s_, pi=pi, xts=xts: emit_proj_piece(s_, pi, xts))
                            )
                        for ci in range(4):
                            c = 4 * qb + ci
                            extra.append(
                                (6 + ci // 2, lambda bb=b + 1, c=c: emit_v2_chunk(bb, c))
                            )
                    pending_epis = emit_attention_qb(b, qb, extra)
            for e in pending_epis:
                e(final=True)

    nc.compile()
    _CACHE["nc"] = nc
    return nc


def _w_arrange(W, cols, bf):
    # [D, CW] -> SBUF layout [128, DCH*CW]: w_sb[p, c*CW+w] = W.T[c*128+p, w]
    wt = np.asarray(W, np.float32)[cols, :].T.astype(bf)
    return np.ascontiguousarray(
        wt.reshape(DCH, 128, CW).transpose(1, 0, 2).reshape(128, DCH * CW)
    )


def make_in_maps(x, mask, Wq, bq, Wk, bk, Wv, bv):
    import ml_dtypes

    bf = ml_dtypes.bfloat16
    x = np.asarray(x, dtype=np.float32)
    xT = np.ascontiguousarray(x.reshape(BS, D).T.astype(bf))
    maskT = np.ascontiguousarray(
        np.asarray(mask, dtype=np.float32)
        .reshape(B, KCH, 128)
        .transpose(2, 0, 1)
        .reshape(128, B * KCH)
    )
    in_maps = []
    for c in range(NC):
        cols = slice(c * CW, (c + 1) * CW)
        in_maps.append(
            {
                "xT": xT,
                "wqT": _w_arrange(Wq, cols, bf),
                "wkT": _w_arrange(Wk, cols, bf),
                "wvT": _w_arrange(Wv, cols, bf),
                "bq": np.ascontiguousarray(np.asarray(bq, np.float32)[cols, None]),
                "bk": np.ascontiguousarray(np.asarray(bk, np.float32)[cols, None]),
                "bv": np.ascontiguousarray(np.asarray(bv, np.float32)[cols, None]),
                "maskT": maskT,
            }
        )
    return in_maps


def assemble(results):
    out = np.empty((BS, D), dtype=np.float32)
    for c in range(NC):
        out[:, c * CW : (c + 1) * CW] = results[c]["h_out"]
    return out.reshape(B, S, D)


def kernel(x, mask, Wq, bq, Wk, bk, Wv, bv, **run_kwargs):
    _ensure_import()
    from concourse.bass_utils import run_bass_kernel_spmd

    nc = build_bass()
    in_maps = make_in_maps(x, mask, Wq, bq, Wk, bk, Wv, bv)
    res = run_bass_kernel_spmd(nc, in_maps, core_ids=list(range(NC)), **run_kwargs)
    _CACHE["last_results"] = res
    return assemble(res.results)
